# revision 13
# baseline (speedup 1.0000x reference)
"""Trainium2 Bass kernel for: relu(1 - beta + x @ W^T).

Shapes (hardcoded): x [4096, 4096] f32, weights [4096, 4096] f32, beta [1] f32.
Output: [4096, 4096] f32.

Strategy: 8 cores as a 4 (batch) x 2 (output) grid. Host pre-transposes x/W to
fp16 so the contraction dim (IN) lands on SBUF partitions with contiguous DMA;
matmuls run fp16 x fp16 -> fp32 PSUM (~2.5e-4 rel err), the ReLU + (1-beta)
bias epilogue reads PSUM on ScalarE/VectorE. Raw Bacc (no Tile) with
hand-rolled semaphores and a minimal exit sequence.

Feature flags (bisectable):
  warmup   — vector memsets a scratch tile, tensor runs NDUMMY dummy matmuls
             on it to spin the PE HAM clock up during the head DMA wait
  head_opt — head-critical loads (w tile 0, x tile 0) on scalar's HWDGE ring
             (earliest main start), x tile 1 on sync; else baseline layout
             (w ring entirely on sync, x kt<2 chunked over scalar+gpsimd)
  seq_last — last pass group-sequential (m outer, kt inner) against a
             prefetched w slice, so only one 256 KB tile's epilogue+store
             remains after the final matmul; else baseline kt-outer last pass

Parameterized sizes so a miniature version can be validated in CoreSim.
"""
import numpy as np

import concourse.bass as bass
import concourse.mybir as mybir
from concourse import bacc

F32 = mybir.dt.float32
F16 = mybir.dt.float16


def build_raw(IN=4096, MB=1024, NO=2048, W_BUFS=12, NDUMMY=20, safe_exit=False,
              warmup=True, head_opt=True, seq_last=True):
    KT = IN // 128          # contraction tiles
    NT = NO // 512          # output-col passes
    MT = MB // 128          # batch-row tiles (psum banks used)
    assert MT <= 8 and MT % 2 == 0 and NT >= 2 and KT >= 2
    NW_RING = (NT - 1) * KT if seq_last else NT * KT  # w tiles via the ring

    nc = bacc.Bacc("TRN2", target_bir_lowering=False, debug=False)
    xT = nc.dram_tensor("xT", [IN, MB], F16, kind="ExternalInput").ap()
    wT = nc.dram_tensor("wT", [IN, NO], F16, kind="ExternalInput").ap()
    beta = nc.dram_tensor("beta", [128, 1], F32, kind="ExternalInput").ap()
    out = nc.dram_tensor("out", [MB, NO], F32, kind="ExternalOutput").ap()

    x_sb = nc.alloc_sbuf_tensor("x_sb", [128, KT, MB], F16).ap()
    w_sb = nc.alloc_sbuf_tensor("w_sb", [128, W_BUFS, 512], F16).ap()
    if seq_last:
        w3_sb = nc.alloc_sbuf_tensor("w3_sb", [128, KT, 512], F16).ap()
    o_sb = nc.alloc_sbuf_tensor("o_sb", [128, 2, MT, 512], F32).ap()
    beta_sb = nc.alloc_sbuf_tensor("beta_sb", [128, 1], F32).ap()
    bias_sb = nc.alloc_sbuf_tensor("bias_sb", [128, 1], F32).ap()
    if warmup:
        warm_sb = nc.alloc_sbuf_tensor("warm_sb", [128, 384], F16).ap()
    ps = nc.alloc_psum_tensor("ps", [128, MT, 512], F32).ap()

    # ---- semaphores ----
    first_sem = None

    def sem(name):
        nonlocal first_sem
        s = nc.alloc_semaphore(name)
        if first_sem is None:
            first_sem = s
        return s

    x_lo = 2 if head_opt else 0
    s_x = {kt: sem(f"s_x{kt}") for kt in range(x_lo, KT)}  # gpsimd x tiles
    if head_opt:
        s_x0a = sem("s_x0a")                         # x tile 0 halves (scalar)
        s_x0b = sem("s_x0b")
        s_x1 = sem("s_x1")                           # x tile 1 (sync)
    else:
        s_xs = [sem("s_xs0"), sem("s_xs1")]          # scalar startup x chunks
    s_w = [sem(f"s_w{s}") for s in range(W_BUFS)]    # w slot arrivals (HWDGE)
    s_wu = sem("s_wu")                               # w tiles consumed (PE, +1)
    s_mm = sem("s_mm")                               # (j,m) accum groups done
    s_eps = sem("s_eps")                             # scalar epilogue ops (+1)
    s_epv = sem("s_epv")                             # vector epilogue ops (+1)
    s_o = [sem("s_o0"), sem("s_o1")]                 # mid-pass store completions
    s_b = sem("s_b")                                 # beta arrival
    s_bias = sem("s_bias")                           # bias computed
    if warmup:
        s_warm = sem("s_warm")                       # warmup scratch memset done
        s_dum = sem("s_dum")                         # warmup dummy MMs done
    if seq_last:
        # last-pass w arrivals, striped over 4 sems to keep counts low (a
        # single counter would reach 16*KT = 512)
        s_w3 = [sem(f"s_wlast{q}") for q in range(4)]
    s_fin = sem("s_fin")                             # scalar+vector final relay
    last_sem = s_fin
    sem_range = range(first_sem.num, last_sem.num + 1)
    # store sems live outside the main range: left to Bacc's defensive
    # full-range reset (after every engine's exit DRAIN), keeping the main
    # semaphore teardown off the store-drain path
    s_oS = sem("s_oS")      # sync-issued last-pass stores (HWDGE)
    s_oG = sem("s_oG")      # gpsimd-issued last-pass stores (SWDGE)

    # x chunk counts for the baseline head (first two k-tiles split for
    # startup latency, interleaved scalar/gpsimd)
    def x_chunks(kt):
        return 4 if kt < 2 else 1

    # number of w DMA chunks for tile index i (early tiles split for latency)
    def w_chunks(i):
        return 2 if i < 2 else 1

    # cumulative inc target for w slot when consuming tile index i
    w_slot_target = [0] * W_BUFS
    w_targets = []
    for i in range(NW_RING):
        sl = i % W_BUFS
        w_slot_target[sl] += 16 * w_chunks(i)
        w_targets.append(w_slot_target[sl])

    # store accounting: only mid-pass stores (gpsimd, 2 DMAs each) carry
    # waited-on semaphores. Last-pass stores are gated only by epilogue sems;
    # data landing before NEFF end is guaranteed by Bacc's exit-sequence
    # per-engine DRAIN, which waits out the issuing engine's DGE queues.
    o_slot_cum = [0, 0]
    o_targets = []                        # cumulative per slot AFTER each pass
    for j in range(NT - 1):
        o_slot_cum[j % 2] += 32
        o_targets.append(o_slot_cum[j % 2])

    # epilogue inc target for (j, m): scalar does even m, vector odd
    def ep_wait(j, m):
        if m % 2 == 0:
            return s_eps, (MT // 2) * j + m // 2 + 1
        return s_epv, (MT // 2) * j + (m - 1) // 2 + 1

    def emit_store_pass(eng, j):
        """Both 4-m halves of pass j as two DMAs (used for j < NT-1)."""
        eng.wait_ge(s_eps, (MT // 2) * (j + 1))
        eng.wait_ge(s_epv, (MT // 2) * (j + 1))
        half = MT // 2
        for h in range(2):
            eng.dma_start(
                out[h * half * 128:(h + 1) * half * 128,
                    j * 512:(j + 1) * 512].rearrange("(m p) c -> p m c", p=128),
                o_sb[:, j % 2, h * half:(h + 1) * half, :],
            ).then_inc(s_o[j % 2], 16)

    def emit_last_store(eng, m, ssem):
        """Single last-pass store for m-tile m (final tiles split in two)."""
        j = NT - 1
        wsem, wval = ep_wait(j, m)
        eng.wait_ge(wsem, wval)
        if m < MT - 2:
            eng.dma_start(
                out[m * 128:(m + 1) * 128, j * 512:(j + 1) * 512],
                o_sb[:, j % 2, m, :],
            ).then_inc(ssem, 16)
        else:
            for ci in range(2):
                eng.dma_start(
                    out[m * 128:(m + 1) * 128,
                        j * 512 + ci * 256:j * 512 + (ci + 1) * 256],
                    o_sb[:, j % 2, m, ci * 256:(ci + 1) * 256],
                ).then_inc(ssem, 16)

    with nc.Block() as block:

        @block.scalar
        def _(scalar: bass.BassEngine):
            if head_opt:
                # head-critical loads on scalar's own HWDGE ring (its main
                # starts ~0.5us before sync's)
                for ci in range(2):
                    scalar.dma_start(
                        w_sb[:, 0, ci * 256:(ci + 1) * 256],
                        wT[0:128, ci * 256:(ci + 1) * 256],
                    ).then_inc(s_w[0], 16)
                scalar.dma_start(
                    x_sb[:, 0, 0:MB // 2], xT[0:128, 0:MB // 2]
                ).then_inc(s_x0a, 16)
                scalar.dma_start(
                    x_sb[:, 0, MB // 2:MB], xT[0:128, MB // 2:MB]
                ).then_inc(s_x0b, 16)
                scalar.dma_start(beta_sb[:], beta[:]).then_inc(s_b, 16)
            else:
                # startup x chunks (odd chunks of first two k-tiles)
                for kt in range(2):
                    nch = x_chunks(kt)
                    cw = MB // nch
                    for ci in range(nch):
                        if ci % 2 == 0:
                            continue
                        scalar.dma_start(
                            x_sb[:, kt, ci * cw:(ci + 1) * cw],
                            xT[kt * 128:(kt + 1) * 128, ci * cw:(ci + 1) * cw],
                        ).then_inc(s_xs[kt], 16)
            for j in range(NT):
                for m in range(0, MT, 2):
                    scalar.wait_ge(s_mm, MT * j + m + 1)
                    if j == 0 and m == 0:
                        scalar.wait_ge(s_bias, 1)
                    if j >= 2:
                        scalar.wait_ge(s_o[j % 2], o_targets[j - 2])
                    scalar.activation(
                        o_sb[:, j % 2, m, :], ps[:, m, :],
                        mybir.ActivationFunctionType.Relu,
                        bias=bias_sb[:], scale=1.0,
                    ).then_inc(s_eps, 1)
            scalar.sem_inc(s_fin, 1)

        @block.sync
        def _(sync: bass.BassEngine):
            if head_opt:
                # x tile 1 (kt=1) in two chunks
                for ci in range(2):
                    sync.dma_start(
                        x_sb[:, 1, ci * (MB // 2):(ci + 1) * (MB // 2)],
                        xT[128:256, ci * (MB // 2):(ci + 1) * (MB // 2)],
                    ).then_inc(s_x1, 16)
            # w ring stream (tile 0 issued by scalar when head_opt)
            for i in range(1 if head_opt else 0, NW_RING):
                j, kt = divmod(i, KT)
                sl = i % W_BUFS
                if i >= W_BUFS:
                    sync.wait_ge(s_wu, i - W_BUFS + 1)
                nch = w_chunks(i)
                cw = 512 // nch
                for ci in range(nch):
                    sync.dma_start(
                        w_sb[:, sl, ci * cw:(ci + 1) * cw],
                        wT[kt * 128:(kt + 1) * 128,
                           j * 512 + ci * cw:j * 512 + (ci + 1) * cw],
                    ).then_inc(s_w[sl], 16)
                if not head_opt and i == 2:
                    # beta load off the critical first-w path
                    sync.dma_start(beta_sb[:], beta[:]).then_inc(s_b, 16)
            if seq_last:
                # last-pass w prefetch (dedicated buffer, no ring waits)
                for kt in range(KT):
                    sync.dma_start(
                        w3_sb[:, kt, :],
                        wT[kt * 128:(kt + 1) * 128, (NT - 1) * 512:NT * 512],
                    ).then_inc(s_w3[kt % 4], 16)
            # last pass, odd m stores (even m on gpsimd in parallel)
            for m in range(1, MT, 2):
                emit_last_store(sync, m, s_oS)

        @block.gpsimd
        def _(gpsimd: bass.BassEngine):
            if head_opt:
                for kt in range(2, KT):
                    gpsimd.dma_start(
                        x_sb[:, kt, :], xT[kt * 128:(kt + 1) * 128, :]
                    ).then_inc(s_x[kt], 16)
            else:
                for kt in range(KT):
                    nch = x_chunks(kt)
                    cw = MB // nch
                    for ci in range(nch):
                        if kt < 2 and ci % 2 == 1:
                            continue  # issued by scalar
                        gpsimd.dma_start(
                            x_sb[:, kt, ci * cw:(ci + 1) * cw],
                            xT[kt * 128:(kt + 1) * 128, ci * cw:(ci + 1) * cw],
                        ).then_inc(s_x[kt], 16)
            for j in range(NT - 1):
                emit_store_pass(gpsimd, j)
            # last pass, even m
            for m in range(0, MT, 2):
                emit_last_store(gpsimd, m, s_oG)
            # teardown: sync with scalar+vector engine clocks (which carry
            # PE's transitively via their s_mm waits), gate on mid-pass store
            # completions, then reset DMA state and clear all kernel
            # semaphores in two instructions.
            gpsimd.wait_ge(s_fin, 2)
            gpsimd.wait_ge(s_o[0], o_slot_cum[0])
            if o_slot_cum[1]:
                gpsimd.wait_ge(s_o[1], o_slot_cum[1])
            if not safe_exit:
                gpsimd.dma_reset(sem_range)
                gpsimd.sem_clear(sem_range)
            # store sems (s_oS/s_oG, outside the cleared range) are zeroed by
            # Bacc's defensive full-range reset, which runs after every
            # engine's exit DRAIN — i.e. after both store queues drain.

        @block.vector
        def _(vector: bass.BassEngine):
            if warmup:
                vector.memset(warm_sb[:], 0.0).then_inc(s_warm, 1)
            vector.wait_ge(s_b, 16)
            vector.tensor_scalar(
                bias_sb[:], beta_sb[:], -1.0, -1.0,
                mybir.AluOpType.mult, mybir.AluOpType.subtract,
            ).then_inc(s_bias, 1)
            for j in range(NT):
                for m in range(1, MT, 2):
                    vector.wait_ge(s_mm, MT * j + m + 1)
                    if j == 0 and m == 1:
                        # self-edge for the race detector: orders the
                        # bias_sb write before this engine's reads
                        vector.wait_ge(s_bias, 1)
                    if j >= 2:
                        vector.wait_ge(s_o[j % 2], o_targets[j - 2])
                    vector.tensor_scalar(
                        o_sb[:, j % 2, m, :], ps[:, m, :], bias_sb[:], 0.0,
                        mybir.AluOpType.add, mybir.AluOpType.max,
                    ).then_inc(s_epv, 1)
            vector.sem_inc(s_fin, 1)

        @block.tensor
        def _(tensor: bass.BassEngine):
            if warmup:
                # dummy matmuls on zeroed scratch: keep the PE busy through
                # the head DMA wait so the HAM clock gate opens (1.2 -> 2.4
                # GHz) before/soon after real matmuls start. Bank 0 garbage
                # is discarded by the first real start=True matmul.
                tensor.wait_ge(s_warm, 1)
                for _ in range(NDUMMY):
                    tensor.matmul(
                        ps[:, 0, 0:256], warm_sb[:, 0:128],
                        warm_sb[:, 128:384], start=True, stop=True,
                    ).then_inc(s_dum, 1)
                # self-wait: publishes the dummies' PSUM writes into the PE
                # clock so downstream s_mm waiters are race-clean vs them
                tensor.wait_ge(s_dum, NDUMMY)
            i = 0
            pending_wu = 0  # w-tile-consumed incs not yet attached (see below)
            for j in range(NT - 1 if seq_last else NT):
                for kt in range(KT):
                    sl = i % W_BUFS
                    tensor.wait_ge(s_w[sl], w_targets[i])
                    if j == 0:
                        if head_opt:
                            if kt == 1:
                                tensor.wait_ge(s_x1, 32)
                            elif kt >= 2:
                                tensor.wait_ge(s_x[kt], 16)
                        else:
                            nch = x_chunks(kt)
                            tensor.wait_ge(s_x[kt], 16 * (nch - nch // 2))
                            if kt < 2:
                                tensor.wait_ge(s_xs[kt], 16 * (nch // 2))
                    for m in range(MT):
                        if head_opt and j == 0 and kt == 0 and m == 0:
                            tensor.wait_ge(s_x0a, 16)
                        if head_opt and j == 0 and kt == 0 and m == MT // 2:
                            tensor.wait_ge(s_x0b, 16)
                        if kt == 0 and j > 0:
                            wsem, wval = ep_wait(j - 1, m)
                            tensor.wait_ge(wsem, wval)
                        mm = tensor.matmul(
                            ps[:, m, :],
                            x_sb[:, kt, m * 128:(m + 1) * 128],
                            w_sb[:, sl, :],
                            start=(kt == 0),
                            stop=(kt == KT - 1),
                        )
                        # One sem update max per instruction. kt==KT-1 MMs
                        # must carry s_mm (epilogue gating, in (j, m) order),
                        # so the w-consumed inc of a pass's last tile is
                        # deferred to the next pass's first MM — safe because
                        # PE completions are pc-monotone.
                        if kt == KT - 1:
                            mm.then_inc(s_mm, 1)
                        elif m == MT - 1:
                            mm.then_inc(s_wu, 1 + pending_wu)
                            pending_wu = 0
                        elif pending_wu:
                            mm.then_inc(s_wu, pending_wu)
                            pending_wu = 0
                    if kt == KT - 1:
                        pending_wu += 1
                    i += 1
            if seq_last:
                # last pass: group-sequential (m outer, kt inner) so each
                # m-tile's epilogue+store overlaps the next 32-MM stream.
                # Full-count gates: partial counts can't prove which tile
                # landed (16 incs may mix transfers); the prefetch finishes
                # ~8us before this pass starts, so they cost nothing.
                j = NT - 1
                for q in range(4):
                    tensor.wait_ge(s_w3[q], 16 * len(range(q, KT, 4)))
                for m in range(MT):
                    wsem, wval = ep_wait(j - 1, m)
                    tensor.wait_ge(wsem, wval)
                    for kt in range(KT):
                        mm = tensor.matmul(
                            ps[:, m, :],
                            x_sb[:, kt, m * 128:(m + 1) * 128],
                            w3_sb[:, kt, :],
                            start=(kt == 0),
                            stop=(kt == KT - 1),
                        )
                        if kt == KT - 1:
                            mm.then_inc(s_mm, 1)
                        elif pending_wu:
                            mm.then_inc(s_wu, pending_wu)
                            pending_wu = 0

    if safe_exit:
        # CoreSim's race detector requires a full barrier before clearing
        nc.sync.drain()
        nc.all_engine_barrier()
        nc.gpsimd.dma_reset(sem_range)
        nc.gpsimd.sem_clear(sem_range)
        # store sems (s_oS/s_oG) left to Bacc's defensive reset; CoreSim
        # never re-executes, and its race detector cannot model DMA-update
        # clocks, so no explicit clear here.
    nc.compile()
    return nc




GRID_B, GRID_O = 4, 2
MB_SHARD, NO_SHARD = 4096 // GRID_B, 4096 // GRID_O

_NC_CACHE = None

# bisect flags for build variants (read once at build)
import os
_WARMUP = os.environ.get("K_WARMUP", "1") == "1"
_HEAD_OPT = os.environ.get("K_HEAD_OPT", "1") == "1"
_SEQ_LAST = os.environ.get("K_SEQ_LAST", "1") == "1"


def _get_nc():
    global _NC_CACHE
    if _NC_CACHE is None:
        _NC_CACHE = build_raw(IN=4096, MB=MB_SHARD, NO=NO_SHARD, W_BUFS=12,
                              warmup=_WARMUP, head_opt=_HEAD_OPT,
                              seq_last=_SEQ_LAST)
    return _NC_CACHE


def kernel(x, weights, beta, _trace=False, _results_out=None):
    from concourse.bass_utils import run_bass_kernel_spmd

    x = np.asarray(x, dtype=np.float32)
    weights = np.asarray(weights, dtype=np.float32)
    beta = np.asarray(beta, dtype=np.float32)

    xT = np.ascontiguousarray(x.T.astype(np.float16))        # [IN, BATCH]
    wT = np.ascontiguousarray(weights.T.astype(np.float16))  # [IN, OUT]
    beta_b = np.ascontiguousarray(
        np.broadcast_to(beta.reshape(1, 1), (128, 1)).astype(np.float32)
    )

    in_maps = []
    for c in range(GRID_B * GRID_O):
        bi, oj = divmod(c, GRID_O)
        in_maps.append({
            "xT": np.ascontiguousarray(xT[:, bi * MB_SHARD:(bi + 1) * MB_SHARD]),
            "wT": np.ascontiguousarray(wT[:, oj * NO_SHARD:(oj + 1) * NO_SHARD]),
            "beta": beta_b,
        })

    nc = _get_nc()
    res = run_bass_kernel_spmd(
        nc, in_maps, core_ids=list(range(8)), trace=_trace,
        trace_cores=list(range(8)) if _trace else None,
    )
    if _results_out is not None:
        _results_out.append(res)

    out = np.empty((4096, 4096), dtype=np.float32)
    for c in range(GRID_B * GRID_O):
        bi, oj = divmod(c, GRID_O)
        out[bi * MB_SHARD:(bi + 1) * MB_SHARD,
            oj * NO_SHARD:(oj + 1) * NO_SHARD] = res.results[c]["out"]
    return out


# revision 15
# speedup vs baseline: 1.1820x; 1.1820x over previous
"""Trainium2 Bass kernel for: relu(1 - beta + x @ W^T).

Shapes (hardcoded): x [4096, 4096] f32, weights [4096, 4096] f32, beta [1] f32.
Output: [4096, 4096] f32.

Strategy: 8 cores as a 4 (batch) x 2 (output) grid. Host pre-transposes x/W to
fp16 so the contraction dim (IN) lands on SBUF partitions with contiguous DMA;
matmuls run fp16 x fp16 -> fp32 PSUM (~2.5e-4 rel err), the ReLU + (1-beta)
bias epilogue reads PSUM on ScalarE/VectorE. Raw Bacc (no Tile) with
hand-rolled semaphores and a minimal exit sequence.

Feature flags (bisectable):
  warmup   — vector memsets a scratch tile, tensor runs NDUMMY dummy matmuls
             on it to spin the PE HAM clock up during the head DMA wait
  head_opt — head-critical loads (w tile 0, x tile 0) on scalar's HWDGE ring
             (earliest main start), x tile 1 on sync; else baseline layout
             (w ring entirely on sync, x kt<2 chunked over scalar+gpsimd)
  seq_last — last pass group-sequential (m outer, kt inner) against a
             prefetched w slice, so only one 256 KB tile's epilogue+store
             remains after the final matmul; else baseline kt-outer last pass

Parameterized sizes so a miniature version can be validated in CoreSim.
"""
import numpy as np

import concourse.bass as bass
import concourse.mybir as mybir
from concourse import bacc

F32 = mybir.dt.float32
F16 = mybir.dt.float16


def build_raw(IN=4096, MB=1024, NO=2048, W_BUFS=12, NDUMMY=20, safe_exit=False,
              warmup=True, head_opt=True, seq_last=True):
    KT = IN // 128          # contraction tiles
    NT = NO // 512          # output-col passes
    MT = MB // 128          # batch-row tiles (psum banks used)
    assert MT <= 8 and MT % 2 == 0 and NT >= 2 and KT >= 2
    NW_RING = (NT - 1) * KT if seq_last else NT * KT  # w tiles via the ring

    nc = bacc.Bacc("TRN2", target_bir_lowering=False, debug=False)
    xT = nc.dram_tensor("xT", [IN, MB], F16, kind="ExternalInput").ap()
    wT = nc.dram_tensor("wT", [IN, NO], F16, kind="ExternalInput").ap()
    beta = nc.dram_tensor("beta", [128, 1], F32, kind="ExternalInput").ap()
    out = nc.dram_tensor("out", [MB, NO], F32, kind="ExternalOutput").ap()

    x_sb = nc.alloc_sbuf_tensor("x_sb", [128, KT, MB], F16).ap()
    w_sb = nc.alloc_sbuf_tensor("w_sb", [128, W_BUFS, 512], F16).ap()
    if seq_last:
        w3_sb = nc.alloc_sbuf_tensor("w3_sb", [128, KT, 512], F16).ap()
    o_sb = nc.alloc_sbuf_tensor("o_sb", [128, 2, MT, 512], F32).ap()
    beta_sb = nc.alloc_sbuf_tensor("beta_sb", [128, 1], F32).ap()
    bias_sb = nc.alloc_sbuf_tensor("bias_sb", [128, 1], F32).ap()
    if warmup:
        warm_sb = nc.alloc_sbuf_tensor("warm_sb", [128, 384], F16).ap()
    ps = nc.alloc_psum_tensor("ps", [128, MT, 512], F32).ap()

    # ---- semaphores ----
    first_sem = None

    def sem(name):
        nonlocal first_sem
        s = nc.alloc_semaphore(name)
        if first_sem is None:
            first_sem = s
        return s

    x_lo = 2 if head_opt else 0
    s_x = {kt: sem(f"s_x{kt}") for kt in range(x_lo, KT)}  # gpsimd x tiles
    if head_opt:
        s_x0a = sem("s_x0a")                         # x tile 0 halves (scalar)
        s_x0b = sem("s_x0b")
        s_x1 = sem("s_x1")                           # x tile 1 (sync)
    else:
        s_xs = [sem("s_xs0"), sem("s_xs1")]          # scalar startup x chunks
    s_w = [sem(f"s_w{s}") for s in range(W_BUFS)]    # w slot arrivals (HWDGE)
    s_wu = sem("s_wu")                               # w tiles consumed (PE, +1)
    s_mm = sem("s_mm")                               # (j,m) accum groups done
    s_eps = sem("s_eps")                             # scalar epilogue ops (+1)
    s_epv = sem("s_epv")                             # vector epilogue ops (+1)
    s_o = [sem("s_o0"), sem("s_o1")]                 # mid-pass store completions
    s_b = sem("s_b")                                 # beta arrival
    s_bias = sem("s_bias")                           # bias computed
    if warmup:
        s_warm = sem("s_warm")                       # warmup scratch memset done
        s_dum = sem("s_dum")                         # warmup dummy MMs done
    if seq_last:
        # last-pass w arrivals, striped over 4 sems to keep counts low (a
        # single counter would reach 16*KT = 512)
        s_w3 = [sem(f"s_wlast{q}") for q in range(4)]
    s_fin = sem("s_fin")                             # scalar+vector final relay
    last_sem = s_fin
    sem_range = range(first_sem.num, last_sem.num + 1)
    # store sems live outside the main range: left to Bacc's defensive
    # full-range reset (after every engine's exit DRAIN), keeping the main
    # semaphore teardown off the store-drain path
    s_oS = sem("s_oS")      # sync-issued last-pass stores (HWDGE)
    s_oG = sem("s_oG")      # gpsimd-issued last-pass stores (SWDGE)

    # x chunk counts for the baseline head (first two k-tiles split for
    # startup latency, interleaved scalar/gpsimd)
    def x_chunks(kt):
        return 4 if kt < 2 else 1

    # number of w DMA chunks for tile index i (early tiles split for latency)
    def w_chunks(i):
        return 2 if i < 2 else 1

    # cumulative inc target for w slot when consuming tile index i
    w_slot_target = [0] * W_BUFS
    w_targets = []
    for i in range(NW_RING):
        sl = i % W_BUFS
        w_slot_target[sl] += 16 * w_chunks(i)
        w_targets.append(w_slot_target[sl])

    # store accounting: only mid-pass stores (gpsimd, 2 DMAs each) carry
    # waited-on semaphores. Last-pass stores are gated only by epilogue sems;
    # data landing before NEFF end is guaranteed by Bacc's exit-sequence
    # per-engine DRAIN, which waits out the issuing engine's DGE queues.
    o_slot_cum = [0, 0]
    o_targets = []                        # cumulative per slot AFTER each pass
    for j in range(NT - 1):
        o_slot_cum[j % 2] += 32
        o_targets.append(o_slot_cum[j % 2])

    # epilogue inc target for (j, m): scalar does even m, vector odd
    def ep_wait(j, m):
        if m % 2 == 0:
            return s_eps, (MT // 2) * j + m // 2 + 1
        return s_epv, (MT // 2) * j + (m - 1) // 2 + 1

    def emit_store_pass(eng, j):
        """Both 4-m halves of pass j as two DMAs (used for j < NT-1)."""
        eng.wait_ge(s_eps, (MT // 2) * (j + 1))
        eng.wait_ge(s_epv, (MT // 2) * (j + 1))
        half = MT // 2
        for h in range(2):
            eng.dma_start(
                out[h * half * 128:(h + 1) * half * 128,
                    j * 512:(j + 1) * 512].rearrange("(m p) c -> p m c", p=128),
                o_sb[:, j % 2, h * half:(h + 1) * half, :],
            ).then_inc(s_o[j % 2], 16)

    def emit_last_store(eng, m, ssem):
        """Single last-pass store for m-tile m (final tiles split in two)."""
        j = NT - 1
        wsem, wval = ep_wait(j, m)
        eng.wait_ge(wsem, wval)
        if m < MT - 2:
            eng.dma_start(
                out[m * 128:(m + 1) * 128, j * 512:(j + 1) * 512],
                o_sb[:, j % 2, m, :],
            ).then_inc(ssem, 16)
        else:
            for ci in range(2):
                eng.dma_start(
                    out[m * 128:(m + 1) * 128,
                        j * 512 + ci * 256:j * 512 + (ci + 1) * 256],
                    o_sb[:, j % 2, m, ci * 256:(ci + 1) * 256],
                ).then_inc(ssem, 16)

    with nc.Block() as block:

        @block.scalar
        def _(scalar: bass.BassEngine):
            if head_opt:
                # head-critical loads on scalar's own HWDGE ring (its main
                # starts ~0.5us before sync's)
                for ci in range(2):
                    scalar.dma_start(
                        w_sb[:, 0, ci * 256:(ci + 1) * 256],
                        wT[0:128, ci * 256:(ci + 1) * 256],
                    ).then_inc(s_w[0], 16)
                scalar.dma_start(
                    x_sb[:, 0, 0:MB // 2], xT[0:128, 0:MB // 2]
                ).then_inc(s_x0a, 16)
                scalar.dma_start(
                    x_sb[:, 0, MB // 2:MB], xT[0:128, MB // 2:MB]
                ).then_inc(s_x0b, 16)
                scalar.dma_start(beta_sb[:], beta[:]).then_inc(s_b, 16)
            else:
                # startup x chunks (odd chunks of first two k-tiles)
                for kt in range(2):
                    nch = x_chunks(kt)
                    cw = MB // nch
                    for ci in range(nch):
                        if ci % 2 == 0:
                            continue
                        scalar.dma_start(
                            x_sb[:, kt, ci * cw:(ci + 1) * cw],
                            xT[kt * 128:(kt + 1) * 128, ci * cw:(ci + 1) * cw],
                        ).then_inc(s_xs[kt], 16)
            for j in range(NT):
                for m in range(0, MT, 2):
                    scalar.wait_ge(s_mm, MT * j + m + 1)
                    if j == 0 and m == 0:
                        scalar.wait_ge(s_bias, 1)
                    if j >= 2:
                        scalar.wait_ge(s_o[j % 2], o_targets[j - 2])
                    scalar.activation(
                        o_sb[:, j % 2, m, :], ps[:, m, :],
                        mybir.ActivationFunctionType.Relu,
                        bias=bias_sb[:], scale=1.0,
                    ).then_inc(s_eps, 1)
            scalar.sem_inc(s_fin, 1)

        @block.sync
        def _(sync: bass.BassEngine):
            if head_opt:
                # x tile 1 (kt=1) in two chunks
                for ci in range(2):
                    sync.dma_start(
                        x_sb[:, 1, ci * (MB // 2):(ci + 1) * (MB // 2)],
                        xT[128:256, ci * (MB // 2):(ci + 1) * (MB // 2)],
                    ).then_inc(s_x1, 16)
            # w ring stream (tile 0 issued by scalar when head_opt)
            for i in range(1 if head_opt else 0, NW_RING):
                j, kt = divmod(i, KT)
                sl = i % W_BUFS
                if i >= W_BUFS:
                    sync.wait_ge(s_wu, i - W_BUFS + 1)
                nch = w_chunks(i)
                cw = 512 // nch
                for ci in range(nch):
                    sync.dma_start(
                        w_sb[:, sl, ci * cw:(ci + 1) * cw],
                        wT[kt * 128:(kt + 1) * 128,
                           j * 512 + ci * cw:j * 512 + (ci + 1) * cw],
                    ).then_inc(s_w[sl], 16)
                if not head_opt and i == 2:
                    # beta load off the critical first-w path
                    sync.dma_start(beta_sb[:], beta[:]).then_inc(s_b, 16)
            if seq_last:
                # last-pass w prefetch (dedicated buffer, no ring waits)
                for kt in range(KT):
                    sync.dma_start(
                        w3_sb[:, kt, :],
                        wT[kt * 128:(kt + 1) * 128, (NT - 1) * 512:NT * 512],
                    ).then_inc(s_w3[kt % 4], 16)
            # last pass, odd m stores (even m on gpsimd in parallel)
            for m in range(1, MT, 2):
                emit_last_store(sync, m, s_oS)

        @block.gpsimd
        def _(gpsimd: bass.BassEngine):
            if head_opt:
                for kt in range(2, KT):
                    gpsimd.dma_start(
                        x_sb[:, kt, :], xT[kt * 128:(kt + 1) * 128, :]
                    ).then_inc(s_x[kt], 16)
            else:
                for kt in range(KT):
                    nch = x_chunks(kt)
                    cw = MB // nch
                    for ci in range(nch):
                        if kt < 2 and ci % 2 == 1:
                            continue  # issued by scalar
                        gpsimd.dma_start(
                            x_sb[:, kt, ci * cw:(ci + 1) * cw],
                            xT[kt * 128:(kt + 1) * 128, ci * cw:(ci + 1) * cw],
                        ).then_inc(s_x[kt], 16)
            for j in range(NT - 1):
                emit_store_pass(gpsimd, j)
            # last pass, even m
            for m in range(0, MT, 2):
                emit_last_store(gpsimd, m, s_oG)
            # teardown: sync with scalar+vector engine clocks (which carry
            # PE's transitively via their s_mm waits), gate on mid-pass store
            # completions, then reset DMA state and clear all kernel
            # semaphores in two instructions.
            gpsimd.wait_ge(s_fin, 2)
            gpsimd.wait_ge(s_o[0], o_slot_cum[0])
            if o_slot_cum[1]:
                gpsimd.wait_ge(s_o[1], o_slot_cum[1])
            if not safe_exit:
                gpsimd.dma_reset(sem_range)
                gpsimd.sem_clear(sem_range)
            # store sems (s_oS/s_oG, outside the cleared range) are zeroed by
            # Bacc's defensive full-range reset, which runs after every
            # engine's exit DRAIN — i.e. after both store queues drain.

        @block.vector
        def _(vector: bass.BassEngine):
            if warmup:
                vector.memset(warm_sb[:], 0.0).then_inc(s_warm, 1)
            vector.wait_ge(s_b, 16)
            vector.tensor_scalar(
                bias_sb[:], beta_sb[:], -1.0, -1.0,
                mybir.AluOpType.mult, mybir.AluOpType.subtract,
            ).then_inc(s_bias, 1)
            for j in range(NT):
                for m in range(1, MT, 2):
                    vector.wait_ge(s_mm, MT * j + m + 1)
                    if j == 0 and m == 1:
                        # self-edge for the race detector: orders the
                        # bias_sb write before this engine's reads
                        vector.wait_ge(s_bias, 1)
                    if j >= 2:
                        vector.wait_ge(s_o[j % 2], o_targets[j - 2])
                    vector.tensor_scalar(
                        o_sb[:, j % 2, m, :], ps[:, m, :], bias_sb[:], 0.0,
                        mybir.AluOpType.add, mybir.AluOpType.max,
                    ).then_inc(s_epv, 1)
            vector.sem_inc(s_fin, 1)

        @block.tensor
        def _(tensor: bass.BassEngine):
            if warmup:
                # dummy matmuls on zeroed scratch: keep the PE busy through
                # the head DMA wait so the HAM clock gate opens (1.2 -> 2.4
                # GHz) before/soon after real matmuls start. Bank 0 garbage
                # is discarded by the first real start=True matmul.
                tensor.wait_ge(s_warm, 1)
                for _ in range(NDUMMY):
                    tensor.matmul(
                        ps[:, 0, 0:256], warm_sb[:, 0:128],
                        warm_sb[:, 128:384], start=True, stop=True,
                    ).then_inc(s_dum, 1)
                # self-wait: publishes the dummies' PSUM writes into the PE
                # clock so downstream s_mm waiters are race-clean vs them
                tensor.wait_ge(s_dum, NDUMMY)
            i = 0
            pending_wu = 0  # w-tile-consumed incs not yet attached (see below)
            for j in range(NT - 1 if seq_last else NT):
                for kt in range(KT):
                    sl = i % W_BUFS
                    tensor.wait_ge(s_w[sl], w_targets[i])
                    if j == 0:
                        if head_opt:
                            if kt == 1:
                                tensor.wait_ge(s_x1, 32)
                            elif kt >= 2:
                                tensor.wait_ge(s_x[kt], 16)
                        else:
                            nch = x_chunks(kt)
                            tensor.wait_ge(s_x[kt], 16 * (nch - nch // 2))
                            if kt < 2:
                                tensor.wait_ge(s_xs[kt], 16 * (nch // 2))
                    for m in range(MT):
                        if head_opt and j == 0 and kt == 0 and m == 0:
                            tensor.wait_ge(s_x0a, 16)
                        if head_opt and j == 0 and kt == 0 and m == MT // 2:
                            tensor.wait_ge(s_x0b, 16)
                        if kt == 0 and j > 0:
                            wsem, wval = ep_wait(j - 1, m)
                            tensor.wait_ge(wsem, wval)
                        mm = tensor.matmul(
                            ps[:, m, :],
                            x_sb[:, kt, m * 128:(m + 1) * 128],
                            w_sb[:, sl, :],
                            start=(kt == 0),
                            stop=(kt == KT - 1),
                        )
                        # One sem update max per instruction. kt==KT-1 MMs
                        # must carry s_mm (epilogue gating, in (j, m) order),
                        # so the w-consumed inc of a pass's last tile is
                        # deferred to the next pass's first MM — safe because
                        # PE completions are pc-monotone.
                        if kt == KT - 1:
                            mm.then_inc(s_mm, 1)
                        elif m == MT - 1:
                            mm.then_inc(s_wu, 1 + pending_wu)
                            pending_wu = 0
                        elif pending_wu:
                            mm.then_inc(s_wu, pending_wu)
                            pending_wu = 0
                    if kt == KT - 1:
                        pending_wu += 1
                    i += 1
            if seq_last:
                # last pass against the prefetched w3 slice. seq_last==1:
                # group-sequential (m outer, kt inner) so each m-tile's
                # epilogue+store overlaps the next 32-MM stream; seq_last==2:
                # baseline kt-outer order (bisect variant).
                # Full-count gates: partial counts can't prove which tile
                # landed (16 incs may mix transfers); the prefetch finishes
                # ~8us before this pass starts, so they cost nothing.
                j = NT - 1
                for q in range(4):
                    tensor.wait_ge(s_w3[q], 16 * len(range(q, KT, 4)))
                if seq_last == 1:
                    loop = [(m, kt) for m in range(MT) for kt in range(KT)]
                else:
                    loop = [(m, kt) for kt in range(KT) for m in range(MT)]
                for m, kt in loop:
                    if kt == 0:
                        wsem, wval = ep_wait(j - 1, m)
                        tensor.wait_ge(wsem, wval)
                    mm = tensor.matmul(
                        ps[:, m, :],
                        x_sb[:, kt, m * 128:(m + 1) * 128],
                        w3_sb[:, kt, :],
                        start=(kt == 0),
                        stop=(kt == KT - 1),
                    )
                    if kt == KT - 1:
                        mm.then_inc(s_mm, 1)
                    elif pending_wu:
                        mm.then_inc(s_wu, pending_wu)
                        pending_wu = 0

    if safe_exit:
        # CoreSim's race detector requires a full barrier before clearing
        nc.sync.drain()
        nc.all_engine_barrier()
        nc.gpsimd.dma_reset(sem_range)
        nc.gpsimd.sem_clear(sem_range)
        # store sems (s_oS/s_oG) left to Bacc's defensive reset; CoreSim
        # never re-executes, and its race detector cannot model DMA-update
        # clocks, so no explicit clear here.
    nc.compile()
    return nc




GRID_B, GRID_O = 4, 2
MB_SHARD, NO_SHARD = 4096 // GRID_B, 4096 // GRID_O

_NC_CACHE = None

# bisect flags for build variants (read once at build)
import os
_WARMUP = os.environ.get("K_WARMUP", "1") == "1"
_HEAD_OPT = os.environ.get("K_HEAD_OPT", "1") == "1"
_SEQ_LAST = int(os.environ.get("K_SEQ_LAST", "1"))


def _get_nc():
    global _NC_CACHE
    if _NC_CACHE is None:
        _NC_CACHE = build_raw(IN=4096, MB=MB_SHARD, NO=NO_SHARD, W_BUFS=12,
                              warmup=_WARMUP, head_opt=_HEAD_OPT,
                              seq_last=_SEQ_LAST)
    return _NC_CACHE


def kernel(x, weights, beta, _trace=False, _results_out=None):
    from concourse.bass_utils import run_bass_kernel_spmd

    x = np.asarray(x, dtype=np.float32)
    weights = np.asarray(weights, dtype=np.float32)
    beta = np.asarray(beta, dtype=np.float32)

    xT = np.ascontiguousarray(x.T.astype(np.float16))        # [IN, BATCH]
    wT = np.ascontiguousarray(weights.T.astype(np.float16))  # [IN, OUT]
    beta_b = np.ascontiguousarray(
        np.broadcast_to(beta.reshape(1, 1), (128, 1)).astype(np.float32)
    )

    in_maps = []
    for c in range(GRID_B * GRID_O):
        bi, oj = divmod(c, GRID_O)
        in_maps.append({
            "xT": np.ascontiguousarray(xT[:, bi * MB_SHARD:(bi + 1) * MB_SHARD]),
            "wT": np.ascontiguousarray(wT[:, oj * NO_SHARD:(oj + 1) * NO_SHARD]),
            "beta": beta_b,
        })

    nc = _get_nc()
    res = run_bass_kernel_spmd(
        nc, in_maps, core_ids=list(range(8)), trace=_trace,
        trace_cores=list(range(8)) if _trace else None,
    )
    if _results_out is not None:
        _results_out.append(res)

    out = np.empty((4096, 4096), dtype=np.float32)
    for c in range(GRID_B * GRID_O):
        bi, oj = divmod(c, GRID_O)
        out[bi * MB_SHARD:(bi + 1) * MB_SHARD,
            oj * NO_SHARD:(oj + 1) * NO_SHARD] = res.results[c]["out"]
    return out


# revision 17
# speedup vs baseline: 1.1847x; 1.0023x over previous
"""Trainium2 Bass kernel for: relu(1 - beta + x @ W^T).

Shapes (hardcoded): x [4096, 4096] f32, weights [4096, 4096] f32, beta [1] f32.
Output: [4096, 4096] f32.

Strategy: 8 cores as a 4 (batch) x 2 (output) grid. Host pre-transposes x/W to
fp16 so the contraction dim (IN) lands on SBUF partitions with contiguous DMA;
matmuls run fp16 x fp16 -> fp32 PSUM (~2.5e-4 rel err), the ReLU + (1-beta)
bias epilogue reads PSUM on ScalarE/VectorE. Raw Bacc (no Tile) with
hand-rolled semaphores and a minimal exit sequence.

Feature flags (bisectable):
  warmup   — vector memsets a scratch tile, tensor runs NDUMMY dummy matmuls
             on it to spin the PE HAM clock up during the head DMA wait
  head_opt — head-critical loads (w tile 0, x tile 0) on scalar's HWDGE ring
             (earliest main start), x tile 1 on sync; else baseline layout
             (w ring entirely on sync, x kt<2 chunked over scalar+gpsimd)
  seq_last — last pass group-sequential (m outer, kt inner) against a
             prefetched w slice, so only one 256 KB tile's epilogue+store
             remains after the final matmul; else baseline kt-outer last pass

Parameterized sizes so a miniature version can be validated in CoreSim.
"""
import numpy as np

import concourse.bass as bass
import concourse.mybir as mybir
from concourse import bacc

F32 = mybir.dt.float32
F16 = mybir.dt.float16


def build_raw(IN=4096, MB=1024, NO=2048, W_BUFS=12, NDUMMY=32, safe_exit=False,
              warmup=True, head_opt=True, seq_last=True):
    KT = IN // 128          # contraction tiles
    NT = NO // 512          # output-col passes
    MT = MB // 128          # batch-row tiles (psum banks used)
    assert MT <= 8 and MT % 2 == 0 and NT >= 2 and KT >= 2
    NW_RING = (NT - 1) * KT if seq_last else NT * KT  # w tiles via the ring

    nc = bacc.Bacc("TRN2", target_bir_lowering=False, debug=False)
    xT = nc.dram_tensor("xT", [IN, MB], F16, kind="ExternalInput").ap()
    wT = nc.dram_tensor("wT", [IN, NO], F16, kind="ExternalInput").ap()
    beta = nc.dram_tensor("beta", [128, 1], F32, kind="ExternalInput").ap()
    out = nc.dram_tensor("out", [MB, NO], F32, kind="ExternalOutput").ap()

    x_sb = nc.alloc_sbuf_tensor("x_sb", [128, KT, MB], F16).ap()
    w_sb = nc.alloc_sbuf_tensor("w_sb", [128, W_BUFS, 512], F16).ap()
    if seq_last:
        w3_sb = nc.alloc_sbuf_tensor("w3_sb", [128, KT, 512], F16).ap()
    o_sb = nc.alloc_sbuf_tensor("o_sb", [128, 2, MT, 512], F32).ap()
    beta_sb = nc.alloc_sbuf_tensor("beta_sb", [128, 1], F32).ap()
    bias_sb = nc.alloc_sbuf_tensor("bias_sb", [128, 1], F32).ap()
    if warmup:
        warm_sb = nc.alloc_sbuf_tensor("warm_sb", [128, 384], F16).ap()
    ps = nc.alloc_psum_tensor("ps", [128, MT, 512], F32).ap()

    # ---- semaphores ----
    first_sem = None

    def sem(name):
        nonlocal first_sem
        s = nc.alloc_semaphore(name)
        if first_sem is None:
            first_sem = s
        return s

    x_lo = 1 if head_opt else 0
    s_x = {kt: sem(f"s_x{kt}") for kt in range(x_lo, KT)}  # gpsimd x tiles
    if head_opt:
        s_x0 = sem("s_x0")                           # x tile 0 (sync)
    else:
        s_xs = [sem("s_xs0"), sem("s_xs1")]          # scalar startup x chunks
    s_w = [sem(f"s_w{s}") for s in range(W_BUFS)]    # w slot arrivals (HWDGE)
    s_wu = sem("s_wu")                               # w tiles consumed (PE, +1)
    s_mm = sem("s_mm")                               # (j,m) accum groups done
    s_eps = sem("s_eps")                             # scalar epilogue ops (+1)
    s_epv = sem("s_epv")                             # vector epilogue ops (+1)
    s_o = [sem("s_o0"), sem("s_o1")]                 # mid-pass store completions
    s_b = sem("s_b")                                 # beta arrival
    s_bias = sem("s_bias")                           # bias computed
    if warmup:
        s_warm = sem("s_warm")                       # warmup scratch memset done
        s_dum = sem("s_dum")                         # warmup dummy MMs done
    if seq_last:
        # last-pass w arrivals, striped over 4 sems to keep counts low (a
        # single counter would reach 16*KT = 512)
        s_w3 = [sem(f"s_wlast{q}") for q in range(4)]
    s_fin = sem("s_fin")                             # scalar+vector final relay
    last_sem = s_fin
    sem_range = range(first_sem.num, last_sem.num + 1)
    # store sems live outside the main range: left to Bacc's defensive
    # full-range reset (after every engine's exit DRAIN), keeping the main
    # semaphore teardown off the store-drain path
    s_oS = sem("s_oS")      # sync-issued last-pass stores (HWDGE)
    s_oG = sem("s_oG")      # gpsimd-issued last-pass stores (SWDGE)

    # x chunk counts for the baseline head (first two k-tiles split for
    # startup latency, interleaved scalar/gpsimd)
    def x_chunks(kt):
        return 4 if kt < 2 else 1

    # number of w DMA chunks for tile index i. Splitting halves transfer
    # latency but doubles sequencer issue time (~700ns per dma_start), so the
    # optimized head never splits.
    def w_chunks(i):
        return 1 if head_opt else (2 if i < 2 else 1)

    # cumulative inc target for w slot when consuming tile index i
    w_slot_target = [0] * W_BUFS
    w_targets = []
    for i in range(NW_RING):
        sl = i % W_BUFS
        w_slot_target[sl] += 16 * w_chunks(i)
        w_targets.append(w_slot_target[sl])

    # store accounting: only mid-pass stores (gpsimd, 2 DMAs each) carry
    # waited-on semaphores. Last-pass stores are gated only by epilogue sems;
    # data landing before NEFF end is guaranteed by Bacc's exit-sequence
    # per-engine DRAIN, which waits out the issuing engine's DGE queues.
    o_slot_cum = [0, 0]
    o_targets = []                        # cumulative per slot AFTER each pass
    for j in range(NT - 1):
        o_slot_cum[j % 2] += 32
        o_targets.append(o_slot_cum[j % 2])

    # epilogue inc target for (j, m): scalar does even m, vector odd
    def ep_wait(j, m):
        if m % 2 == 0:
            return s_eps, (MT // 2) * j + m // 2 + 1
        return s_epv, (MT // 2) * j + (m - 1) // 2 + 1

    def emit_store_pass(eng, j):
        """Both 4-m halves of pass j as two DMAs (used for j < NT-1)."""
        eng.wait_ge(s_eps, (MT // 2) * (j + 1))
        eng.wait_ge(s_epv, (MT // 2) * (j + 1))
        half = MT // 2
        for h in range(2):
            eng.dma_start(
                out[h * half * 128:(h + 1) * half * 128,
                    j * 512:(j + 1) * 512].rearrange("(m p) c -> p m c", p=128),
                o_sb[:, j % 2, h * half:(h + 1) * half, :],
            ).then_inc(s_o[j % 2], 16)

    def emit_last_store(eng, m, ssem):
        """Single last-pass store for m-tile m (final tiles split in two)."""
        j = NT - 1
        wsem, wval = ep_wait(j, m)
        eng.wait_ge(wsem, wval)
        if m < MT - 2:
            eng.dma_start(
                out[m * 128:(m + 1) * 128, j * 512:(j + 1) * 512],
                o_sb[:, j % 2, m, :],
            ).then_inc(ssem, 16)
        else:
            for ci in range(2):
                eng.dma_start(
                    out[m * 128:(m + 1) * 128,
                        j * 512 + ci * 256:j * 512 + (ci + 1) * 256],
                    o_sb[:, j % 2, m, ci * 256:(ci + 1) * 256],
                ).then_inc(ssem, 16)

    with nc.Block() as block:

        @block.scalar
        def _(scalar: bass.BassEngine):
            if head_opt:
                # w tile 0 on scalar's own HWDGE ring (its main starts ~0.5us
                # before sync's); one unsplit DMA — issue cost dominates
                scalar.dma_start(w_sb[:, 0, :], wT[0:128, 0:512]
                                 ).then_inc(s_w[0], 16)
                scalar.dma_start(beta_sb[:], beta[:]).then_inc(s_b, 16)
            else:
                # startup x chunks (odd chunks of first two k-tiles)
                for kt in range(2):
                    nch = x_chunks(kt)
                    cw = MB // nch
                    for ci in range(nch):
                        if ci % 2 == 0:
                            continue
                        scalar.dma_start(
                            x_sb[:, kt, ci * cw:(ci + 1) * cw],
                            xT[kt * 128:(kt + 1) * 128, ci * cw:(ci + 1) * cw],
                        ).then_inc(s_xs[kt], 16)
            for j in range(NT):
                for m in range(0, MT, 2):
                    scalar.wait_ge(s_mm, MT * j + m + 1)
                    if j == 0 and m == 0:
                        scalar.wait_ge(s_bias, 1)
                    if j >= 2:
                        scalar.wait_ge(s_o[j % 2], o_targets[j - 2])
                    scalar.activation(
                        o_sb[:, j % 2, m, :], ps[:, m, :],
                        mybir.ActivationFunctionType.Relu,
                        bias=bias_sb[:], scale=1.0,
                    ).then_inc(s_eps, 1)
            scalar.sem_inc(s_fin, 1)

        @block.sync
        def _(sync: bass.BassEngine):
            if head_opt:
                # x tile 0, one unsplit DMA
                sync.dma_start(x_sb[:, 0, :], xT[0:128, :]).then_inc(s_x0, 16)
            # w ring stream (tile 0 issued by scalar when head_opt)
            for i in range(1 if head_opt else 0, NW_RING):
                j, kt = divmod(i, KT)
                sl = i % W_BUFS
                if i >= W_BUFS:
                    sync.wait_ge(s_wu, i - W_BUFS + 1)
                nch = w_chunks(i)
                cw = 512 // nch
                for ci in range(nch):
                    sync.dma_start(
                        w_sb[:, sl, ci * cw:(ci + 1) * cw],
                        wT[kt * 128:(kt + 1) * 128,
                           j * 512 + ci * cw:j * 512 + (ci + 1) * cw],
                    ).then_inc(s_w[sl], 16)
                if not head_opt and i == 2:
                    # beta load off the critical first-w path
                    sync.dma_start(beta_sb[:], beta[:]).then_inc(s_b, 16)
            if seq_last:
                # last-pass w prefetch (dedicated buffer, no ring waits)
                for kt in range(KT):
                    sync.dma_start(
                        w3_sb[:, kt, :],
                        wT[kt * 128:(kt + 1) * 128, (NT - 1) * 512:NT * 512],
                    ).then_inc(s_w3[kt % 4], 16)
            # last pass, odd m stores (even m on gpsimd in parallel)
            for m in range(1, MT, 2):
                emit_last_store(sync, m, s_oS)

        @block.gpsimd
        def _(gpsimd: bass.BassEngine):
            if head_opt:
                for kt in range(1, KT):
                    gpsimd.dma_start(
                        x_sb[:, kt, :], xT[kt * 128:(kt + 1) * 128, :]
                    ).then_inc(s_x[kt], 16)
            else:
                for kt in range(KT):
                    nch = x_chunks(kt)
                    cw = MB // nch
                    for ci in range(nch):
                        if kt < 2 and ci % 2 == 1:
                            continue  # issued by scalar
                        gpsimd.dma_start(
                            x_sb[:, kt, ci * cw:(ci + 1) * cw],
                            xT[kt * 128:(kt + 1) * 128, ci * cw:(ci + 1) * cw],
                        ).then_inc(s_x[kt], 16)
            for j in range(NT - 1):
                emit_store_pass(gpsimd, j)
            # last pass, even m
            for m in range(0, MT, 2):
                emit_last_store(gpsimd, m, s_oG)
            # teardown: sync with scalar+vector engine clocks (which carry
            # PE's transitively via their s_mm waits), gate on mid-pass store
            # completions, then reset DMA state and clear all kernel
            # semaphores in two instructions.
            gpsimd.wait_ge(s_fin, 2)
            gpsimd.wait_ge(s_o[0], o_slot_cum[0])
            if o_slot_cum[1]:
                gpsimd.wait_ge(s_o[1], o_slot_cum[1])
            if not safe_exit:
                gpsimd.dma_reset(sem_range)
                gpsimd.sem_clear(sem_range)
            # store sems (s_oS/s_oG, outside the cleared range) are zeroed by
            # Bacc's defensive full-range reset, which runs after every
            # engine's exit DRAIN — i.e. after both store queues drain.

        @block.vector
        def _(vector: bass.BassEngine):
            if warmup:
                vector.memset(warm_sb[:], 0.0).then_inc(s_warm, 1)
            vector.wait_ge(s_b, 16)
            vector.tensor_scalar(
                bias_sb[:], beta_sb[:], -1.0, -1.0,
                mybir.AluOpType.mult, mybir.AluOpType.subtract,
            ).then_inc(s_bias, 1)
            for j in range(NT):
                for m in range(1, MT, 2):
                    vector.wait_ge(s_mm, MT * j + m + 1)
                    if j == 0 and m == 1:
                        # self-edge for the race detector: orders the
                        # bias_sb write before this engine's reads
                        vector.wait_ge(s_bias, 1)
                    if j >= 2:
                        vector.wait_ge(s_o[j % 2], o_targets[j - 2])
                    vector.tensor_scalar(
                        o_sb[:, j % 2, m, :], ps[:, m, :], bias_sb[:], 0.0,
                        mybir.AluOpType.add, mybir.AluOpType.max,
                    ).then_inc(s_epv, 1)
            vector.sem_inc(s_fin, 1)

        @block.tensor
        def _(tensor: bass.BassEngine):
            if warmup:
                # dummy matmuls on zeroed scratch: keep the PE busy through
                # the head DMA wait so the HAM clock gate opens (1.2 -> 2.4
                # GHz) before/soon after real matmuls start. Bank 0 garbage
                # is discarded by the first real start=True matmul.
                tensor.wait_ge(s_warm, 1)
                for _ in range(NDUMMY):
                    tensor.matmul(
                        ps[:, 0, 0:128], warm_sb[:, 0:128],
                        warm_sb[:, 128:256], start=True, stop=True,
                    ).then_inc(s_dum, 1)
                # self-wait: publishes the dummies' PSUM writes into the PE
                # clock so downstream s_mm waiters are race-clean vs them
                tensor.wait_ge(s_dum, NDUMMY)
            i = 0
            pending_wu = 0  # w-tile-consumed incs not yet attached (see below)
            for j in range(NT - 1 if seq_last else NT):
                for kt in range(KT):
                    sl = i % W_BUFS
                    tensor.wait_ge(s_w[sl], w_targets[i])
                    if j == 0:
                        if head_opt:
                            if kt >= 1:
                                tensor.wait_ge(s_x[kt], 16)
                        else:
                            nch = x_chunks(kt)
                            tensor.wait_ge(s_x[kt], 16 * (nch - nch // 2))
                            if kt < 2:
                                tensor.wait_ge(s_xs[kt], 16 * (nch // 2))
                    for m in range(MT):
                        if head_opt and j == 0 and kt == 0 and m == 0:
                            tensor.wait_ge(s_x0, 16)
                        if kt == 0 and j > 0:
                            wsem, wval = ep_wait(j - 1, m)
                            tensor.wait_ge(wsem, wval)
                        mm = tensor.matmul(
                            ps[:, m, :],
                            x_sb[:, kt, m * 128:(m + 1) * 128],
                            w_sb[:, sl, :],
                            start=(kt == 0),
                            stop=(kt == KT - 1),
                        )
                        # One sem update max per instruction. kt==KT-1 MMs
                        # must carry s_mm (epilogue gating, in (j, m) order),
                        # so the w-consumed inc of a pass's last tile is
                        # deferred to the next pass's first MM — safe because
                        # PE completions are pc-monotone.
                        if kt == KT - 1:
                            mm.then_inc(s_mm, 1)
                        elif m == MT - 1:
                            mm.then_inc(s_wu, 1 + pending_wu)
                            pending_wu = 0
                        elif pending_wu:
                            mm.then_inc(s_wu, pending_wu)
                            pending_wu = 0
                    if kt == KT - 1:
                        pending_wu += 1
                    i += 1
            if seq_last:
                # last pass against the prefetched w3 slice. seq_last==1:
                # group-sequential (m outer, kt inner) so each m-tile's
                # epilogue+store overlaps the next 32-MM stream; seq_last==2:
                # baseline kt-outer order (bisect variant).
                # Full-count gates: partial counts can't prove which tile
                # landed (16 incs may mix transfers); the prefetch finishes
                # ~8us before this pass starts, so they cost nothing.
                j = NT - 1
                for q in range(4):
                    tensor.wait_ge(s_w3[q], 16 * len(range(q, KT, 4)))
                if seq_last == 1:
                    loop = [(m, kt) for m in range(MT) for kt in range(KT)]
                elif seq_last == 3:
                    loop = [(2 * mp + m, kt) for mp in range(MT // 2)
                            for kt in range(KT) for m in range(2)]
                else:
                    loop = [(m, kt) for kt in range(KT) for m in range(MT)]
                for m, kt in loop:
                    if kt == 0:
                        wsem, wval = ep_wait(j - 1, m)
                        tensor.wait_ge(wsem, wval)
                    mm = tensor.matmul(
                        ps[:, m, :],
                        x_sb[:, kt, m * 128:(m + 1) * 128],
                        w3_sb[:, kt, :],
                        start=(kt == 0),
                        stop=(kt == KT - 1),
                    )
                    if kt == KT - 1:
                        mm.then_inc(s_mm, 1)
                    elif pending_wu:
                        mm.then_inc(s_wu, pending_wu)
                        pending_wu = 0

    if safe_exit:
        # CoreSim's race detector requires a full barrier before clearing
        nc.sync.drain()
        nc.all_engine_barrier()
        nc.gpsimd.dma_reset(sem_range)
        nc.gpsimd.sem_clear(sem_range)
        # store sems (s_oS/s_oG) left to Bacc's defensive reset; CoreSim
        # never re-executes, and its race detector cannot model DMA-update
        # clocks, so no explicit clear here.
    nc.compile()
    return nc




GRID_B, GRID_O = 4, 2
MB_SHARD, NO_SHARD = 4096 // GRID_B, 4096 // GRID_O

_NC_CACHE = None

# bisect flags for build variants (read once at build)
import os
_WARMUP = os.environ.get("K_WARMUP", "1") == "1"
_HEAD_OPT = os.environ.get("K_HEAD_OPT", "1") == "1"
_SEQ_LAST = int(os.environ.get("K_SEQ_LAST", "1"))


def _get_nc():
    global _NC_CACHE
    if _NC_CACHE is None:
        _NC_CACHE = build_raw(IN=4096, MB=MB_SHARD, NO=NO_SHARD, W_BUFS=12,
                              warmup=_WARMUP, head_opt=_HEAD_OPT,
                              seq_last=_SEQ_LAST)
    return _NC_CACHE


def kernel(x, weights, beta, _trace=False, _results_out=None):
    from concourse.bass_utils import run_bass_kernel_spmd

    x = np.asarray(x, dtype=np.float32)
    weights = np.asarray(weights, dtype=np.float32)
    beta = np.asarray(beta, dtype=np.float32)

    xT = np.ascontiguousarray(x.T.astype(np.float16))        # [IN, BATCH]
    wT = np.ascontiguousarray(weights.T.astype(np.float16))  # [IN, OUT]
    beta_b = np.ascontiguousarray(
        np.broadcast_to(beta.reshape(1, 1), (128, 1)).astype(np.float32)
    )

    in_maps = []
    for c in range(GRID_B * GRID_O):
        bi, oj = divmod(c, GRID_O)
        in_maps.append({
            "xT": np.ascontiguousarray(xT[:, bi * MB_SHARD:(bi + 1) * MB_SHARD]),
            "wT": np.ascontiguousarray(wT[:, oj * NO_SHARD:(oj + 1) * NO_SHARD]),
            "beta": beta_b,
        })

    nc = _get_nc()
    res = run_bass_kernel_spmd(
        nc, in_maps, core_ids=list(range(8)), trace=_trace,
        trace_cores=list(range(8)) if _trace else None,
    )
    if _results_out is not None:
        _results_out.append(res)

    out = np.empty((4096, 4096), dtype=np.float32)
    for c in range(GRID_B * GRID_O):
        bi, oj = divmod(c, GRID_O)
        out[bi * MB_SHARD:(bi + 1) * MB_SHARD,
            oj * NO_SHARD:(oj + 1) * NO_SHARD] = res.results[c]["out"]
    return out


# revision 18
# speedup vs baseline: 1.2129x; 1.0238x over previous
"""Trainium2 Bass kernel for: relu(1 - beta + x @ W^T).

Shapes (hardcoded): x [4096, 4096] f32, weights [4096, 4096] f32, beta [1] f32.
Output: [4096, 4096] f32.

Strategy: 8 cores as a 4 (batch) x 2 (output) grid. Host pre-transposes x/W to
fp16 so the contraction dim (IN) lands on SBUF partitions with contiguous DMA;
matmuls run fp16 x fp16 -> fp32 PSUM (~2.5e-4 rel err), the ReLU + (1-beta)
bias epilogue reads PSUM on ScalarE/VectorE. Raw Bacc (no Tile) with
hand-rolled semaphores and a minimal exit sequence.

Feature flags (bisectable):
  warmup   — vector memsets a scratch tile, tensor runs NDUMMY dummy matmuls
             on it to spin the PE HAM clock up during the head DMA wait
  head_opt — head-critical loads (w tile 0, x tile 0) on scalar's HWDGE ring
             (earliest main start), x tile 1 on sync; else baseline layout
             (w ring entirely on sync, x kt<2 chunked over scalar+gpsimd)
  seq_last — last pass group-sequential (m outer, kt inner) against a
             prefetched w slice, so only one 256 KB tile's epilogue+store
             remains after the final matmul; else baseline kt-outer last pass

Parameterized sizes so a miniature version can be validated in CoreSim.
"""
import numpy as np

import concourse.bass as bass
import concourse.mybir as mybir
from concourse import bacc

F32 = mybir.dt.float32
F16 = mybir.dt.float16


def build_raw(IN=4096, MB=1024, NO=2048, W_BUFS=12, NDUMMY=38, safe_exit=False,
              warmup=True, head_opt=True, seq_last=True):
    KT = IN // 128          # contraction tiles
    NT = NO // 512          # output-col passes
    MT = MB // 128          # batch-row tiles (psum banks used)
    assert MT <= 8 and MT % 2 == 0 and NT >= 2 and KT >= 2
    NW_RING = (NT - 1) * KT if seq_last else NT * KT  # w tiles via the ring

    nc = bacc.Bacc("TRN2", target_bir_lowering=False, debug=False)
    xT = nc.dram_tensor("xT", [IN, MB], F16, kind="ExternalInput").ap()
    wT = nc.dram_tensor("wT", [IN, NO], F16, kind="ExternalInput").ap()
    beta = nc.dram_tensor("beta", [128, 1], F32, kind="ExternalInput").ap()
    out = nc.dram_tensor("out", [MB, NO], F32, kind="ExternalOutput").ap()

    x_sb = nc.alloc_sbuf_tensor("x_sb", [128, KT, MB], F16).ap()
    w_sb = nc.alloc_sbuf_tensor("w_sb", [128, W_BUFS, 512], F16).ap()
    if seq_last:
        w3_sb = nc.alloc_sbuf_tensor("w3_sb", [128, KT, 512], F16).ap()
    o_sb = nc.alloc_sbuf_tensor("o_sb", [128, 2, MT, 512], F32).ap()
    beta_sb = nc.alloc_sbuf_tensor("beta_sb", [128, 1], F32).ap()
    bias_sb = nc.alloc_sbuf_tensor("bias_sb", [128, 1], F32).ap()
    if warmup:
        warm_sb = nc.alloc_sbuf_tensor("warm_sb", [128, 384], F16).ap()
    ps = nc.alloc_psum_tensor("ps", [128, MT, 512], F32).ap()

    # ---- semaphores ----
    first_sem = None

    def sem(name):
        nonlocal first_sem
        s = nc.alloc_semaphore(name)
        if first_sem is None:
            first_sem = s
        return s

    x_lo = 1 if head_opt else 0
    s_x = {kt: sem(f"s_x{kt}") for kt in range(x_lo, KT)}  # gpsimd x tiles
    if head_opt:
        s_x0 = sem("s_x0")                           # x tile 0 (sync)
    else:
        s_xs = [sem("s_xs0"), sem("s_xs1")]          # scalar startup x chunks
    s_w = [sem(f"s_w{s}") for s in range(W_BUFS)]    # w slot arrivals (HWDGE)
    s_wu = sem("s_wu")                               # w tiles consumed (PE, +1)
    s_mm = sem("s_mm")                               # (j,m) accum groups done
    s_eps = sem("s_eps")                             # scalar epilogue ops (+1)
    s_epv = sem("s_epv")                             # vector epilogue ops (+1)
    s_o = [sem("s_o0"), sem("s_o1")]                 # mid-pass store completions
    s_b = sem("s_b")                                 # beta arrival
    s_bias = sem("s_bias")                           # bias computed
    if warmup:
        s_warm = sem("s_warm")                       # warmup scratch memset done
        s_dum = sem("s_dum")                         # warmup dummy MMs done
    if seq_last:
        # last-pass w arrivals, striped over 4 sems to keep counts low (a
        # single counter would reach 16*KT = 512)
        s_w3 = [sem(f"s_wlast{q}") for q in range(4)]
    s_fin = sem("s_fin")                             # scalar+vector final relay
    last_sem = s_fin
    sem_range = range(first_sem.num, last_sem.num + 1)
    # store sems live outside the main range: left to Bacc's defensive
    # full-range reset (after every engine's exit DRAIN), keeping the main
    # semaphore teardown off the store-drain path
    s_oS = sem("s_oS")      # sync-issued last-pass stores (HWDGE)
    s_oG = sem("s_oG")      # gpsimd-issued last-pass stores (SWDGE)

    # x chunk counts for the baseline head (first two k-tiles split for
    # startup latency, interleaved scalar/gpsimd)
    def x_chunks(kt):
        return 4 if kt < 2 else 1

    # number of w DMA chunks for tile index i. Splitting halves transfer
    # latency but doubles sequencer issue time (~700ns per dma_start), so the
    # optimized head never splits.
    def w_chunks(i):
        return 1 if head_opt else (2 if i < 2 else 1)

    # cumulative inc target for w slot when consuming tile index i
    w_slot_target = [0] * W_BUFS
    w_targets = []
    for i in range(NW_RING):
        sl = i % W_BUFS
        w_slot_target[sl] += 16 * w_chunks(i)
        w_targets.append(w_slot_target[sl])

    # store accounting: only mid-pass stores (gpsimd, 2 DMAs each) carry
    # waited-on semaphores. Last-pass stores are gated only by epilogue sems;
    # data landing before NEFF end is guaranteed by Bacc's exit-sequence
    # per-engine DRAIN, which waits out the issuing engine's DGE queues.
    o_slot_cum = [0, 0]
    o_targets = []                        # cumulative per slot AFTER each pass
    for j in range(NT - 1):
        o_slot_cum[j % 2] += 32
        o_targets.append(o_slot_cum[j % 2])

    # epilogue inc target for (j, m): scalar does even m, vector odd
    def ep_wait(j, m):
        if m % 2 == 0:
            return s_eps, (MT // 2) * j + m // 2 + 1
        return s_epv, (MT // 2) * j + (m - 1) // 2 + 1

    def emit_store_pass(eng, j):
        """Both 4-m halves of pass j as two DMAs (used for j < NT-1)."""
        eng.wait_ge(s_eps, (MT // 2) * (j + 1))
        eng.wait_ge(s_epv, (MT // 2) * (j + 1))
        half = MT // 2
        for h in range(2):
            eng.dma_start(
                out[h * half * 128:(h + 1) * half * 128,
                    j * 512:(j + 1) * 512].rearrange("(m p) c -> p m c", p=128),
                o_sb[:, j % 2, h * half:(h + 1) * half, :],
            ).then_inc(s_o[j % 2], 16)

    def emit_last_store(eng, m, ssem):
        """Single last-pass store for m-tile m (one unsplit DMA: the ~700ns
        per-dma_start sequencer issue cost dominates the transfer split)."""
        j = NT - 1
        wsem, wval = ep_wait(j, m)
        eng.wait_ge(wsem, wval)
        eng.dma_start(
            out[m * 128:(m + 1) * 128, j * 512:(j + 1) * 512],
            o_sb[:, j % 2, m, :],
        ).then_inc(ssem, 16)

    with nc.Block() as block:

        @block.scalar
        def _(scalar: bass.BassEngine):
            if head_opt:
                # w tile 0 on scalar's own HWDGE ring (its main starts ~0.5us
                # before sync's); one unsplit DMA — issue cost dominates
                scalar.dma_start(w_sb[:, 0, :], wT[0:128, 0:512]
                                 ).then_inc(s_w[0], 16)
                scalar.dma_start(beta_sb[:], beta[:]).then_inc(s_b, 16)
            else:
                # startup x chunks (odd chunks of first two k-tiles)
                for kt in range(2):
                    nch = x_chunks(kt)
                    cw = MB // nch
                    for ci in range(nch):
                        if ci % 2 == 0:
                            continue
                        scalar.dma_start(
                            x_sb[:, kt, ci * cw:(ci + 1) * cw],
                            xT[kt * 128:(kt + 1) * 128, ci * cw:(ci + 1) * cw],
                        ).then_inc(s_xs[kt], 16)
            for j in range(NT):
                for m in range(0, MT, 2):
                    scalar.wait_ge(s_mm, MT * j + m + 1)
                    if j == 0 and m == 0:
                        scalar.wait_ge(s_bias, 1)
                    if j >= 2:
                        scalar.wait_ge(s_o[j % 2], o_targets[j - 2])
                    scalar.activation(
                        o_sb[:, j % 2, m, :], ps[:, m, :],
                        mybir.ActivationFunctionType.Relu,
                        bias=bias_sb[:], scale=1.0,
                    ).then_inc(s_eps, 1)
            scalar.sem_inc(s_fin, 1)

        @block.sync
        def _(sync: bass.BassEngine):
            if head_opt:
                # x tile 0, one unsplit DMA
                sync.dma_start(x_sb[:, 0, :], xT[0:128, :]).then_inc(s_x0, 16)
            # w ring stream (tile 0 issued by scalar when head_opt),
            # with the last-pass w3 prefetch interleaved one-per-three ring
            # tiles: each dma_start costs ~700ns of sequencer issue time, so
            # a bunched 32-DMA prefetch at the end of the ring would finish
            # ~15us after the last pass needs it (measured 4.5us PE stall)
            w3_i = 0

            def issue_w3():
                nonlocal w3_i
                sync.dma_start(
                    w3_sb[:, w3_i, :],
                    wT[w3_i * 128:(w3_i + 1) * 128, (NT - 1) * 512:NT * 512],
                ).then_inc(s_w3[w3_i % 4], 16)
                w3_i += 1

            for i in range(1 if head_opt else 0, NW_RING):
                j, kt = divmod(i, KT)
                sl = i % W_BUFS
                if i >= W_BUFS:
                    sync.wait_ge(s_wu, i - W_BUFS + 1)
                nch = w_chunks(i)
                cw = 512 // nch
                for ci in range(nch):
                    sync.dma_start(
                        w_sb[:, sl, ci * cw:(ci + 1) * cw],
                        wT[kt * 128:(kt + 1) * 128,
                           j * 512 + ci * cw:j * 512 + (ci + 1) * cw],
                    ).then_inc(s_w[sl], 16)
                if not head_opt and i == 2:
                    # beta load off the critical first-w path
                    sync.dma_start(beta_sb[:], beta[:]).then_inc(s_b, 16)
                if seq_last and i >= W_BUFS and (i - W_BUFS) % 3 == 0 \
                        and w3_i < KT:
                    issue_w3()
            if seq_last:
                while w3_i < KT:
                    issue_w3()
            # last pass, odd m stores (even m on gpsimd in parallel)
            for m in range(1, MT, 2):
                emit_last_store(sync, m, s_oS)

        @block.gpsimd
        def _(gpsimd: bass.BassEngine):
            if head_opt:
                for kt in range(1, KT):
                    gpsimd.dma_start(
                        x_sb[:, kt, :], xT[kt * 128:(kt + 1) * 128, :]
                    ).then_inc(s_x[kt], 16)
            else:
                for kt in range(KT):
                    nch = x_chunks(kt)
                    cw = MB // nch
                    for ci in range(nch):
                        if kt < 2 and ci % 2 == 1:
                            continue  # issued by scalar
                        gpsimd.dma_start(
                            x_sb[:, kt, ci * cw:(ci + 1) * cw],
                            xT[kt * 128:(kt + 1) * 128, ci * cw:(ci + 1) * cw],
                        ).then_inc(s_x[kt], 16)
            for j in range(NT - 1):
                emit_store_pass(gpsimd, j)
            # last pass, even m
            for m in range(0, MT, 2):
                emit_last_store(gpsimd, m, s_oG)
            # teardown: sync with scalar+vector engine clocks (which carry
            # PE's transitively via their s_mm waits), gate on mid-pass store
            # completions, then reset DMA state and clear all kernel
            # semaphores in two instructions.
            gpsimd.wait_ge(s_fin, 2)
            gpsimd.wait_ge(s_o[0], o_slot_cum[0])
            if o_slot_cum[1]:
                gpsimd.wait_ge(s_o[1], o_slot_cum[1])
            if not safe_exit:
                gpsimd.dma_reset(sem_range)
                gpsimd.sem_clear(sem_range)
            # store sems (s_oS/s_oG, outside the cleared range) are zeroed by
            # Bacc's defensive full-range reset, which runs after every
            # engine's exit DRAIN — i.e. after both store queues drain.

        @block.vector
        def _(vector: bass.BassEngine):
            if warmup:
                vector.memset(warm_sb[:], 0.0).then_inc(s_warm, 1)
            vector.wait_ge(s_b, 16)
            vector.tensor_scalar(
                bias_sb[:], beta_sb[:], -1.0, -1.0,
                mybir.AluOpType.mult, mybir.AluOpType.subtract,
            ).then_inc(s_bias, 1)
            for j in range(NT):
                for m in range(1, MT, 2):
                    vector.wait_ge(s_mm, MT * j + m + 1)
                    if j == 0 and m == 1:
                        # self-edge for the race detector: orders the
                        # bias_sb write before this engine's reads
                        vector.wait_ge(s_bias, 1)
                    if j >= 2:
                        vector.wait_ge(s_o[j % 2], o_targets[j - 2])
                    vector.tensor_scalar(
                        o_sb[:, j % 2, m, :], ps[:, m, :], bias_sb[:], 0.0,
                        mybir.AluOpType.add, mybir.AluOpType.max,
                    ).then_inc(s_epv, 1)
            vector.sem_inc(s_fin, 1)

        @block.tensor
        def _(tensor: bass.BassEngine):
            if warmup:
                # dummy matmuls on zeroed scratch: keep the PE busy through
                # the head DMA wait so the HAM clock gate opens (1.2 -> 2.4
                # GHz) before/soon after real matmuls start. Bank 0 garbage
                # is discarded by the first real start=True matmul.
                tensor.wait_ge(s_warm, 1)
                for _ in range(NDUMMY):
                    tensor.matmul(
                        ps[:, 0, 0:128], warm_sb[:, 0:128],
                        warm_sb[:, 128:256], start=True, stop=True,
                    ).then_inc(s_dum, 1)
                # self-wait: publishes the dummies' PSUM writes into the PE
                # clock so downstream s_mm waiters are race-clean vs them
                tensor.wait_ge(s_dum, NDUMMY)
            i = 0
            pending_wu = 0  # w-tile-consumed incs not yet attached (see below)
            for j in range(NT - 1 if seq_last else NT):
                for kt in range(KT):
                    sl = i % W_BUFS
                    tensor.wait_ge(s_w[sl], w_targets[i])
                    if j == 0:
                        if head_opt:
                            if kt >= 1:
                                tensor.wait_ge(s_x[kt], 16)
                        else:
                            nch = x_chunks(kt)
                            tensor.wait_ge(s_x[kt], 16 * (nch - nch // 2))
                            if kt < 2:
                                tensor.wait_ge(s_xs[kt], 16 * (nch // 2))
                    for m in range(MT):
                        if head_opt and j == 0 and kt == 0 and m == 0:
                            tensor.wait_ge(s_x0, 16)
                        if kt == 0 and j > 0:
                            wsem, wval = ep_wait(j - 1, m)
                            tensor.wait_ge(wsem, wval)
                        mm = tensor.matmul(
                            ps[:, m, :],
                            x_sb[:, kt, m * 128:(m + 1) * 128],
                            w_sb[:, sl, :],
                            start=(kt == 0),
                            stop=(kt == KT - 1),
                        )
                        # One sem update max per instruction. kt==KT-1 MMs
                        # must carry s_mm (epilogue gating, in (j, m) order),
                        # so the w-consumed inc of a pass's last tile is
                        # deferred to the next pass's first MM — safe because
                        # PE completions are pc-monotone.
                        if kt == KT - 1:
                            mm.then_inc(s_mm, 1)
                        elif m == MT - 1:
                            mm.then_inc(s_wu, 1 + pending_wu)
                            pending_wu = 0
                        elif pending_wu:
                            mm.then_inc(s_wu, pending_wu)
                            pending_wu = 0
                    if kt == KT - 1:
                        pending_wu += 1
                    i += 1
            if seq_last:
                # last pass against the prefetched w3 slice. seq_last==1:
                # group-sequential (m outer, kt inner) so each m-tile's
                # epilogue+store overlaps the next 32-MM stream; seq_last==2:
                # baseline kt-outer order (bisect variant).
                # Full-count gates: partial counts can't prove which tile
                # landed (16 incs may mix transfers); the prefetch finishes
                # ~8us before this pass starts, so they cost nothing.
                j = NT - 1
                for q in range(4):
                    tensor.wait_ge(s_w3[q], 16 * len(range(q, KT, 4)))
                if seq_last == 1:
                    loop = [(m, kt) for m in range(MT) for kt in range(KT)]
                elif seq_last == 3:
                    loop = [(2 * mp + m, kt) for mp in range(MT // 2)
                            for kt in range(KT) for m in range(2)]
                else:
                    loop = [(m, kt) for kt in range(KT) for m in range(MT)]
                for m, kt in loop:
                    if kt == 0:
                        wsem, wval = ep_wait(j - 1, m)
                        tensor.wait_ge(wsem, wval)
                    mm = tensor.matmul(
                        ps[:, m, :],
                        x_sb[:, kt, m * 128:(m + 1) * 128],
                        w3_sb[:, kt, :],
                        start=(kt == 0),
                        stop=(kt == KT - 1),
                    )
                    if kt == KT - 1:
                        mm.then_inc(s_mm, 1)
                    elif pending_wu:
                        mm.then_inc(s_wu, pending_wu)
                        pending_wu = 0

    if safe_exit:
        # CoreSim's race detector requires a full barrier before clearing
        nc.sync.drain()
        nc.all_engine_barrier()
        nc.gpsimd.dma_reset(sem_range)
        nc.gpsimd.sem_clear(sem_range)
        # store sems (s_oS/s_oG) left to Bacc's defensive reset; CoreSim
        # never re-executes, and its race detector cannot model DMA-update
        # clocks, so no explicit clear here.
    nc.compile()
    return nc




GRID_B, GRID_O = 4, 2
MB_SHARD, NO_SHARD = 4096 // GRID_B, 4096 // GRID_O

_NC_CACHE = None

# bisect flags for build variants (read once at build)
import os
_WARMUP = os.environ.get("K_WARMUP", "1") == "1"
_HEAD_OPT = os.environ.get("K_HEAD_OPT", "1") == "1"
_SEQ_LAST = int(os.environ.get("K_SEQ_LAST", "1"))


def _get_nc():
    global _NC_CACHE
    if _NC_CACHE is None:
        _NC_CACHE = build_raw(IN=4096, MB=MB_SHARD, NO=NO_SHARD, W_BUFS=12,
                              warmup=_WARMUP, head_opt=_HEAD_OPT,
                              seq_last=_SEQ_LAST)
    return _NC_CACHE


def kernel(x, weights, beta, _trace=False, _results_out=None):
    from concourse.bass_utils import run_bass_kernel_spmd

    x = np.asarray(x, dtype=np.float32)
    weights = np.asarray(weights, dtype=np.float32)
    beta = np.asarray(beta, dtype=np.float32)

    xT = np.ascontiguousarray(x.T.astype(np.float16))        # [IN, BATCH]
    wT = np.ascontiguousarray(weights.T.astype(np.float16))  # [IN, OUT]
    beta_b = np.ascontiguousarray(
        np.broadcast_to(beta.reshape(1, 1), (128, 1)).astype(np.float32)
    )

    in_maps = []
    for c in range(GRID_B * GRID_O):
        bi, oj = divmod(c, GRID_O)
        in_maps.append({
            "xT": np.ascontiguousarray(xT[:, bi * MB_SHARD:(bi + 1) * MB_SHARD]),
            "wT": np.ascontiguousarray(wT[:, oj * NO_SHARD:(oj + 1) * NO_SHARD]),
            "beta": beta_b,
        })

    nc = _get_nc()
    res = run_bass_kernel_spmd(
        nc, in_maps, core_ids=list(range(8)), trace=_trace,
        trace_cores=list(range(8)) if _trace else None,
    )
    if _results_out is not None:
        _results_out.append(res)

    out = np.empty((4096, 4096), dtype=np.float32)
    for c in range(GRID_B * GRID_O):
        bi, oj = divmod(c, GRID_O)
        out[bi * MB_SHARD:(bi + 1) * MB_SHARD,
            oj * NO_SHARD:(oj + 1) * NO_SHARD] = res.results[c]["out"]
    return out


# revision 20
# speedup vs baseline: 1.2189x; 1.0049x over previous
"""Trainium2 Bass kernel for: relu(1 - beta + x @ W^T).

Shapes (hardcoded): x [4096, 4096] f32, weights [4096, 4096] f32, beta [1] f32.
Output: [4096, 4096] f32.

Strategy: 8 cores as a 4 (batch) x 2 (output) grid. Host pre-transposes x/W to
fp16 so the contraction dim (IN) lands on SBUF partitions with contiguous DMA;
matmuls run fp16 x fp16 -> fp32 PSUM (~2.5e-4 rel err), the ReLU + (1-beta)
bias epilogue reads PSUM on ScalarE/VectorE. Raw Bacc (no Tile) with
hand-rolled semaphores and a minimal exit sequence.

Feature flags (bisectable):
  warmup   — vector memsets a scratch tile, tensor runs NDUMMY dummy matmuls
             on it to spin the PE HAM clock up during the head DMA wait
  head_opt — head-critical loads (w tile 0, x tile 0) on scalar's HWDGE ring
             (earliest main start), x tile 1 on sync; else baseline layout
             (w ring entirely on sync, x kt<2 chunked over scalar+gpsimd)
  seq_last — last pass group-sequential (m outer, kt inner) against a
             prefetched w slice, so only one 256 KB tile's epilogue+store
             remains after the final matmul; else baseline kt-outer last pass

Parameterized sizes so a miniature version can be validated in CoreSim.
"""
import numpy as np

import concourse.bass as bass
import concourse.mybir as mybir
from concourse import bacc

F32 = mybir.dt.float32
F16 = mybir.dt.float16


def build_raw(IN=4096, MB=1024, NO=2048, W_BUFS=16, NDUMMY=46, safe_exit=False,
              warmup=True, head_opt=True, seq_last=True):
    KT = IN // 128          # contraction tiles
    NT = NO // 512          # output-col passes
    MT = MB // 128          # batch-row tiles (psum banks used)
    assert MT <= 8 and MT % 2 == 0 and NT >= 2 and KT >= 2
    NW_RING = (NT - 1) * KT if seq_last else NT * KT  # w tiles via the ring

    nc = bacc.Bacc("TRN2", target_bir_lowering=False, debug=False)
    xT = nc.dram_tensor("xT", [IN, MB], F16, kind="ExternalInput").ap()
    wT = nc.dram_tensor("wT", [IN, NO], F16, kind="ExternalInput").ap()
    beta = nc.dram_tensor("beta", [128, 1], F32, kind="ExternalInput").ap()
    out = nc.dram_tensor("out", [MB, NO], F32, kind="ExternalOutput").ap()

    x_sb = nc.alloc_sbuf_tensor("x_sb", [128, KT, MB], F16).ap()
    w_sb = nc.alloc_sbuf_tensor("w_sb", [128, W_BUFS, 512], F16).ap()
    if seq_last:
        w3_sb = nc.alloc_sbuf_tensor("w3_sb", [128, KT, 512], F16).ap()
    o_sb = nc.alloc_sbuf_tensor("o_sb", [128, 2, MT, 512], F32).ap()
    beta_sb = nc.alloc_sbuf_tensor("beta_sb", [128, 1], F32).ap()
    bias_sb = nc.alloc_sbuf_tensor("bias_sb", [128, 1], F32).ap()
    if warmup:
        warm_sb = nc.alloc_sbuf_tensor("warm_sb", [128, 384], F16).ap()
    ps = nc.alloc_psum_tensor("ps", [128, MT, 512], F32).ap()

    # ---- semaphores ----
    first_sem = None

    def sem(name):
        nonlocal first_sem
        s = nc.alloc_semaphore(name)
        if first_sem is None:
            first_sem = s
        return s

    x_lo = 1 if head_opt else 0
    s_x = {kt: sem(f"s_x{kt}") for kt in range(x_lo, KT)}  # gpsimd x tiles
    if head_opt:
        s_x0 = sem("s_x0")                           # x tile 0 (sync)
    else:
        s_xs = [sem("s_xs0"), sem("s_xs1")]          # scalar startup x chunks
    s_w = [sem(f"s_w{s}") for s in range(W_BUFS)]    # w slot arrivals (HWDGE)
    s_wu = sem("s_wu")                               # w tiles consumed (PE, +1)
    s_mm = sem("s_mm")                               # (j,m) accum groups done
    s_eps = sem("s_eps")                             # scalar epilogue ops (+1)
    s_epv = sem("s_epv")                             # vector epilogue ops (+1)
    s_o = [sem("s_o0"), sem("s_o1")]                 # mid-pass store completions
    s_b = sem("s_b")                                 # beta arrival
    s_bias = sem("s_bias")                           # bias computed
    if warmup:
        s_warm = sem("s_warm")                       # warmup scratch memset done
        s_dum = sem("s_dum")                         # warmup dummy MMs done
    if seq_last:
        # last-pass w arrivals, striped over 4 sems to keep counts low (a
        # single counter would reach 16*KT = 512)
        s_w3 = [sem(f"s_wlast{q}") for q in range(4)]
    s_fin = sem("s_fin")                             # scalar+vector final relay
    last_sem = s_fin
    sem_range = range(first_sem.num, last_sem.num + 1)
    # store sems live outside the main range: left to Bacc's defensive
    # full-range reset (after every engine's exit DRAIN), keeping the main
    # semaphore teardown off the store-drain path
    s_oS = sem("s_oS")      # sync-issued last-pass stores (HWDGE)
    s_oG = sem("s_oG")      # gpsimd-issued last-pass stores (SWDGE)

    # x chunk counts for the baseline head (first two k-tiles split for
    # startup latency, interleaved scalar/gpsimd)
    def x_chunks(kt):
        return 4 if kt < 2 else 1

    # number of w DMA chunks for tile index i. Splitting halves transfer
    # latency but doubles sequencer issue time (~700ns per dma_start), so the
    # optimized head never splits.
    def w_chunks(i):
        return 1 if head_opt else (2 if i < 2 else 1)

    # cumulative inc target for w slot when consuming tile index i
    w_slot_target = [0] * W_BUFS
    w_targets = []
    for i in range(NW_RING):
        sl = i % W_BUFS
        w_slot_target[sl] += 16 * w_chunks(i)
        w_targets.append(w_slot_target[sl])

    # store accounting: only mid-pass stores (gpsimd, 2 DMAs each) carry
    # waited-on semaphores. Last-pass stores are gated only by epilogue sems;
    # data landing before NEFF end is guaranteed by Bacc's exit-sequence
    # per-engine DRAIN, which waits out the issuing engine's DGE queues.
    o_slot_cum = [0, 0]
    o_targets = []                        # cumulative per slot AFTER each pass
    for j in range(NT - 1):
        o_slot_cum[j % 2] += 32
        o_targets.append(o_slot_cum[j % 2])

    # epilogue inc target for (j, m): scalar does even m, vector odd
    def ep_wait(j, m):
        if m % 2 == 0:
            return s_eps, (MT // 2) * j + m // 2 + 1
        return s_epv, (MT // 2) * j + (m - 1) // 2 + 1

    def emit_store_pass(eng, j):
        """Both 4-m halves of pass j as two DMAs (used for j < NT-1)."""
        eng.wait_ge(s_eps, (MT // 2) * (j + 1))
        eng.wait_ge(s_epv, (MT // 2) * (j + 1))
        half = MT // 2
        for h in range(2):
            eng.dma_start(
                out[h * half * 128:(h + 1) * half * 128,
                    j * 512:(j + 1) * 512].rearrange("(m p) c -> p m c", p=128),
                o_sb[:, j % 2, h * half:(h + 1) * half, :],
            ).then_inc(s_o[j % 2], 16)

    def emit_last_store(eng, m, ssem):
        """Single last-pass store for m-tile m (one unsplit DMA: the ~700ns
        per-dma_start sequencer issue cost dominates the transfer split)."""
        j = NT - 1
        wsem, wval = ep_wait(j, m)
        eng.wait_ge(wsem, wval)
        eng.dma_start(
            out[m * 128:(m + 1) * 128, j * 512:(j + 1) * 512],
            o_sb[:, j % 2, m, :],
        ).then_inc(ssem, 16)

    with nc.Block() as block:

        @block.scalar
        def _(scalar: bass.BassEngine):
            if head_opt:
                # w tile 0 on scalar's own HWDGE ring (its main starts ~0.5us
                # before sync's); one unsplit DMA — issue cost dominates
                scalar.dma_start(w_sb[:, 0, :], wT[0:128, 0:512]
                                 ).then_inc(s_w[0], 16)
                scalar.dma_start(beta_sb[:], beta[:]).then_inc(s_b, 16)
                # ring tiles 1-2 issued here in parallel with sync's stream
                # (scalar idles until the first epilogue anyway)
                for i in (1, 2):
                    if i < NW_RING:
                        scalar.dma_start(
                            w_sb[:, i, :], wT[i * 128:(i + 1) * 128, 0:512]
                        ).then_inc(s_w[i], 16)
            else:
                # startup x chunks (odd chunks of first two k-tiles)
                for kt in range(2):
                    nch = x_chunks(kt)
                    cw = MB // nch
                    for ci in range(nch):
                        if ci % 2 == 0:
                            continue
                        scalar.dma_start(
                            x_sb[:, kt, ci * cw:(ci + 1) * cw],
                            xT[kt * 128:(kt + 1) * 128, ci * cw:(ci + 1) * cw],
                        ).then_inc(s_xs[kt], 16)
            for j in range(NT):
                for m in range(0, MT, 2):
                    scalar.wait_ge(s_mm, MT * j + m + 1)
                    if j == 0 and m == 0:
                        scalar.wait_ge(s_bias, 1)
                    if j >= 2:
                        scalar.wait_ge(s_o[j % 2], o_targets[j - 2])
                    scalar.activation(
                        o_sb[:, j % 2, m, :], ps[:, m, :],
                        mybir.ActivationFunctionType.Relu,
                        bias=bias_sb[:], scale=1.0,
                    ).then_inc(s_eps, 1)
            scalar.sem_inc(s_fin, 1)

        @block.sync
        def _(sync: bass.BassEngine):
            if head_opt:
                # x tile 0, one unsplit DMA
                sync.dma_start(x_sb[:, 0, :], xT[0:128, :]).then_inc(s_x0, 16)
            # w ring stream (tile 0 issued by scalar when head_opt),
            # with the last-pass w3 prefetch interleaved one-per-three ring
            # tiles: each dma_start costs ~700ns of sequencer issue time, so
            # a bunched 32-DMA prefetch at the end of the ring would finish
            # ~15us after the last pass needs it (measured 4.5us PE stall)
            w3_i = 0

            def issue_w3():
                nonlocal w3_i
                sync.dma_start(
                    w3_sb[:, w3_i, :],
                    wT[w3_i * 128:(w3_i + 1) * 128, (NT - 1) * 512:NT * 512],
                ).then_inc(s_w3[w3_i % 4], 16)
                w3_i += 1

            for i in range(3 if head_opt else 0, NW_RING):
                j, kt = divmod(i, KT)
                sl = i % W_BUFS
                if i >= W_BUFS:
                    sync.wait_ge(s_wu, i - W_BUFS + 1)
                nch = w_chunks(i)
                cw = 512 // nch
                for ci in range(nch):
                    sync.dma_start(
                        w_sb[:, sl, ci * cw:(ci + 1) * cw],
                        wT[kt * 128:(kt + 1) * 128,
                           j * 512 + ci * cw:j * 512 + (ci + 1) * cw],
                    ).then_inc(s_w[sl], 16)
                if not head_opt and i == 2:
                    # beta load off the critical first-w path
                    sync.dma_start(beta_sb[:], beta[:]).then_inc(s_b, 16)
                if seq_last and i >= W_BUFS and (i - W_BUFS) % 2 == 0 \
                        and w3_i < KT:
                    issue_w3()
            if seq_last:
                while w3_i < KT:
                    issue_w3()
            # last pass, odd m stores (even m on gpsimd in parallel)
            for m in range(1, MT, 2):
                emit_last_store(sync, m, s_oS)

        @block.gpsimd
        def _(gpsimd: bass.BassEngine):
            if head_opt:
                for kt in range(1, KT):
                    gpsimd.dma_start(
                        x_sb[:, kt, :], xT[kt * 128:(kt + 1) * 128, :]
                    ).then_inc(s_x[kt], 16)
            else:
                for kt in range(KT):
                    nch = x_chunks(kt)
                    cw = MB // nch
                    for ci in range(nch):
                        if kt < 2 and ci % 2 == 1:
                            continue  # issued by scalar
                        gpsimd.dma_start(
                            x_sb[:, kt, ci * cw:(ci + 1) * cw],
                            xT[kt * 128:(kt + 1) * 128, ci * cw:(ci + 1) * cw],
                        ).then_inc(s_x[kt], 16)
            for j in range(NT - 1):
                emit_store_pass(gpsimd, j)
            # last pass, even m
            for m in range(0, MT, 2):
                emit_last_store(gpsimd, m, s_oG)
            # teardown: sync with scalar+vector engine clocks (which carry
            # PE's transitively via their s_mm waits), gate on mid-pass store
            # completions, then reset DMA state and clear all kernel
            # semaphores in two instructions.
            gpsimd.wait_ge(s_fin, 2)
            gpsimd.wait_ge(s_o[0], o_slot_cum[0])
            if o_slot_cum[1]:
                gpsimd.wait_ge(s_o[1], o_slot_cum[1])
            if not safe_exit:
                gpsimd.dma_reset(sem_range)
                gpsimd.sem_clear(sem_range)
            # store sems (s_oS/s_oG, outside the cleared range) are zeroed by
            # Bacc's defensive full-range reset, which runs after every
            # engine's exit DRAIN — i.e. after both store queues drain.

        @block.vector
        def _(vector: bass.BassEngine):
            if warmup:
                vector.memset(warm_sb[:], 0.0).then_inc(s_warm, 1)
            vector.wait_ge(s_b, 16)
            vector.tensor_scalar(
                bias_sb[:], beta_sb[:], -1.0, -1.0,
                mybir.AluOpType.mult, mybir.AluOpType.subtract,
            ).then_inc(s_bias, 1)
            for j in range(NT):
                for m in range(1, MT, 2):
                    vector.wait_ge(s_mm, MT * j + m + 1)
                    if j == 0 and m == 1:
                        # self-edge for the race detector: orders the
                        # bias_sb write before this engine's reads
                        vector.wait_ge(s_bias, 1)
                    if j >= 2:
                        vector.wait_ge(s_o[j % 2], o_targets[j - 2])
                    vector.tensor_scalar(
                        o_sb[:, j % 2, m, :], ps[:, m, :], bias_sb[:], 0.0,
                        mybir.AluOpType.add, mybir.AluOpType.max,
                    ).then_inc(s_epv, 1)
            vector.sem_inc(s_fin, 1)

        @block.tensor
        def _(tensor: bass.BassEngine):
            if warmup:
                # dummy matmuls on zeroed scratch: keep the PE busy through
                # the head DMA wait so the HAM clock gate opens (1.2 -> 2.4
                # GHz) before/soon after real matmuls start. Bank 0 garbage
                # is discarded by the first real start=True matmul.
                tensor.wait_ge(s_warm, 1)
                for _ in range(NDUMMY):
                    tensor.matmul(
                        ps[:, 0, 0:128], warm_sb[:, 0:128],
                        warm_sb[:, 128:256], start=True, stop=True,
                    ).then_inc(s_dum, 1)
                # self-wait: publishes the dummies' PSUM writes into the PE
                # clock so downstream s_mm waiters are race-clean vs them
                tensor.wait_ge(s_dum, NDUMMY)
            i = 0
            pending_wu = 0  # w-tile-consumed incs not yet attached (see below)
            for j in range(NT - 1 if seq_last else NT):
                for kt in range(KT):
                    sl = i % W_BUFS
                    tensor.wait_ge(s_w[sl], w_targets[i])
                    if j == 0:
                        if head_opt:
                            if kt >= 1:
                                tensor.wait_ge(s_x[kt], 16)
                        else:
                            nch = x_chunks(kt)
                            tensor.wait_ge(s_x[kt], 16 * (nch - nch // 2))
                            if kt < 2:
                                tensor.wait_ge(s_xs[kt], 16 * (nch // 2))
                    for m in range(MT):
                        if head_opt and j == 0 and kt == 0 and m == 0:
                            tensor.wait_ge(s_x0, 16)
                        if kt == 0 and j > 0:
                            wsem, wval = ep_wait(j - 1, m)
                            tensor.wait_ge(wsem, wval)
                        mm = tensor.matmul(
                            ps[:, m, :],
                            x_sb[:, kt, m * 128:(m + 1) * 128],
                            w_sb[:, sl, :],
                            start=(kt == 0),
                            stop=(kt == KT - 1),
                        )
                        # One sem update max per instruction. kt==KT-1 MMs
                        # must carry s_mm (epilogue gating, in (j, m) order),
                        # so the w-consumed inc of a pass's last tile is
                        # deferred to the next pass's first MM — safe because
                        # PE completions are pc-monotone.
                        if kt == KT - 1:
                            mm.then_inc(s_mm, 1)
                        elif m == MT - 1:
                            mm.then_inc(s_wu, 1 + pending_wu)
                            pending_wu = 0
                        elif pending_wu:
                            mm.then_inc(s_wu, pending_wu)
                            pending_wu = 0
                    if kt == KT - 1:
                        pending_wu += 1
                    i += 1
            if seq_last:
                # last pass against the prefetched w3 slice. seq_last==1:
                # group-sequential (m outer, kt inner) so each m-tile's
                # epilogue+store overlaps the next 32-MM stream; seq_last==2:
                # baseline kt-outer order (bisect variant).
                # Full-count gates: partial counts can't prove which tile
                # landed (16 incs may mix transfers); the prefetch finishes
                # ~8us before this pass starts, so they cost nothing.
                j = NT - 1
                for q in range(4):
                    tensor.wait_ge(s_w3[q], 16 * len(range(q, KT, 4)))
                if seq_last == 1:
                    loop = [(m, kt) for m in range(MT) for kt in range(KT)]
                elif seq_last == 3:
                    loop = [(2 * mp + m, kt) for mp in range(MT // 2)
                            for kt in range(KT) for m in range(2)]
                else:
                    loop = [(m, kt) for kt in range(KT) for m in range(MT)]
                for m, kt in loop:
                    if kt == 0:
                        wsem, wval = ep_wait(j - 1, m)
                        tensor.wait_ge(wsem, wval)
                    mm = tensor.matmul(
                        ps[:, m, :],
                        x_sb[:, kt, m * 128:(m + 1) * 128],
                        w3_sb[:, kt, :],
                        start=(kt == 0),
                        stop=(kt == KT - 1),
                    )
                    if kt == KT - 1:
                        mm.then_inc(s_mm, 1)
                    elif pending_wu:
                        mm.then_inc(s_wu, pending_wu)
                        pending_wu = 0

    if safe_exit:
        # CoreSim's race detector requires a full barrier before clearing
        nc.sync.drain()
        nc.all_engine_barrier()
        nc.gpsimd.dma_reset(sem_range)
        nc.gpsimd.sem_clear(sem_range)
        # store sems (s_oS/s_oG) left to Bacc's defensive reset; CoreSim
        # never re-executes, and its race detector cannot model DMA-update
        # clocks, so no explicit clear here.
    nc.compile()
    return nc




GRID_B, GRID_O = 4, 2
MB_SHARD, NO_SHARD = 4096 // GRID_B, 4096 // GRID_O

_NC_CACHE = None

# bisect flags for build variants (read once at build)
import os
_WARMUP = os.environ.get("K_WARMUP", "1") == "1"
_HEAD_OPT = os.environ.get("K_HEAD_OPT", "1") == "1"
_SEQ_LAST = int(os.environ.get("K_SEQ_LAST", "1"))


def _get_nc():
    global _NC_CACHE
    if _NC_CACHE is None:
        _NC_CACHE = build_raw(IN=4096, MB=MB_SHARD, NO=NO_SHARD, W_BUFS=16,
                              warmup=_WARMUP, head_opt=_HEAD_OPT,
                              seq_last=_SEQ_LAST)
    return _NC_CACHE


def kernel(x, weights, beta, _trace=False, _results_out=None):
    from concourse.bass_utils import run_bass_kernel_spmd

    x = np.asarray(x, dtype=np.float32)
    weights = np.asarray(weights, dtype=np.float32)
    beta = np.asarray(beta, dtype=np.float32)

    xT = np.ascontiguousarray(x.T.astype(np.float16))        # [IN, BATCH]
    wT = np.ascontiguousarray(weights.T.astype(np.float16))  # [IN, OUT]
    beta_b = np.ascontiguousarray(
        np.broadcast_to(beta.reshape(1, 1), (128, 1)).astype(np.float32)
    )

    in_maps = []
    for c in range(GRID_B * GRID_O):
        bi, oj = divmod(c, GRID_O)
        in_maps.append({
            "xT": np.ascontiguousarray(xT[:, bi * MB_SHARD:(bi + 1) * MB_SHARD]),
            "wT": np.ascontiguousarray(wT[:, oj * NO_SHARD:(oj + 1) * NO_SHARD]),
            "beta": beta_b,
        })

    nc = _get_nc()
    res = run_bass_kernel_spmd(
        nc, in_maps, core_ids=list(range(8)), trace=_trace,
        trace_cores=list(range(8)) if _trace else None,
    )
    if _results_out is not None:
        _results_out.append(res)

    out = np.empty((4096, 4096), dtype=np.float32)
    for c in range(GRID_B * GRID_O):
        bi, oj = divmod(c, GRID_O)
        out[bi * MB_SHARD:(bi + 1) * MB_SHARD,
            oj * NO_SHARD:(oj + 1) * NO_SHARD] = res.results[c]["out"]
    return out
